# revision 1
# baseline (speedup 1.0000x reference)
"""Trainium2 Bass kernel for a pre-norm transformer block (B=4, N=1024, C=1024,
16 heads, MLP hidden 4096), SPMD across 8 NeuronCores.

Sharding: core = (b, s) with b = batch element (4), s = query-half (2).
Each core computes the block for 512 query tokens of one batch element:
  - LN1 + K/V projections over the full 1024-token sequence of its batch
    element (duplicated within the batch pair - avoids all collectives),
  - Q projection + attention + proj + residual + LN2 + MLP for its 512 rows.

Everything on-device is channel-major (channels on partitions, tokens on the
free dim) so no on-device transposes are needed; the host passes x.T and
pre-transposed weights. LayerNorm gains/biases are folded into the following
matmul weights/biases on the host; the softmax scale is folded into the Q
weights. Softmax uses no max-subtraction (logits are O(1) by construction),
which makes the softmax1 "+1" denominator term exact and free; the
denominator itself comes from a ones-column appended to V inside the P@V
matmul.

Precision: main matmuls in float32r (full PE rate, ~1e-4 matmul rel err);
attention S/P/O and the MLP hidden activations in bf16; everything else fp32.
"""

from contextlib import ExitStack

import numpy as np
import ml_dtypes

import concourse.bass as bass
import concourse.mybir as mybir
import concourse.tile as tile
from concourse import bacc

P = 128
B, N, C = 4, 1024, 1024
H, D = 16, 64
HD = 4 * C
NQ = N // 2          # query tokens per core
NK = N               # key/value tokens per core
CT = C // P          # 8 channel tiles
KB = NK // P         # 8 key-token tiles
SCALE = D ** (-0.5)
EPS = 1e-5

F32 = mybir.dt.float32
F32R = mybir.dt.float32r
BF16 = mybir.dt.bfloat16
AF = mybir.ActivationFunctionType
ALU = mybir.AluOpType


def _pb(ap, p):
    """Partition-broadcast: view a [1, ...] AP as [p, ...] with 0 partition
    stride (legal for DMA reads)."""
    return bass.AP(tensor=ap.tensor, offset=ap.offset,
                   ap=[[0, p]] + [list(x) for x in ap.ap[1:]])


def build_nc(stop_after=None):
    nc = _build_body(stop_after)
    nc.compile()
    return nc


def _build_body(stop_after=None):
    nc = bacc.Bacc("TRN2", target_bir_lowering=False, debug=False, num_devices=8)

    # ---- DRAM I/O (per core) ----
    xT = nc.dram_tensor("xT", [C, NK], F32R, kind="ExternalInput")
    ones_d = nc.dram_tensor("ones_d", [P], F32R, kind="ExternalInput")
    xTqb = nc.dram_tensor("xTqb", [C, NQ], F32, kind="ExternalInput")      # x.T[:, q] + proj_b
    maskT = nc.dram_tensor("maskT", [NK, NQ], BF16, kind="ExternalInput")  # 1-mask, transposed
    qkv_wT = nc.dram_tensor("qkv_wT", [C, 3 * C], BF16, kind="ExternalInput")
    qkvb_qk = nc.dram_tensor("qkvb_qk", [2 * C], F32, kind="ExternalInput")
    qkvb_v = nc.dram_tensor("qkvb_v", [1, C], BF16, kind="ExternalInput")
    proj_wT = nc.dram_tensor("proj_wT", [C, C], BF16, kind="ExternalInput")
    fc1_wT = nc.dram_tensor("fc1_wT", [C, HD], BF16, kind="ExternalInput")
    fc1b = nc.dram_tensor("fc1b", [HD], F32, kind="ExternalInput")
    fc2_wT = nc.dram_tensor("fc2_wT", [HD, C], BF16, kind="ExternalInput")
    fc2b = nc.dram_tensor("fc2b", [C], F32, kind="ExternalInput")
    outT = nc.dram_tensor("outT", [C, NQ], F32, kind="ExternalOutput")

    with tile.TileContext(nc) as tc, ExitStack() as ctx:
        persist = ctx.enter_context(tc.tile_pool(name="persist", bufs=1))
        hpool = ctx.enter_context(tc.tile_pool(name="hpool", bufs=1))
        xstream = ctx.enter_context(tc.tile_pool(name="xstream", bufs=4))
        sqpool = ctx.enter_context(tc.tile_pool(name="sqpool", bufs=2))
        wpool = ctx.enter_context(tc.tile_pool(name="wpool", bufs=2))
        vwpool = ctx.enter_context(tc.tile_pool(name="vwpool", bufs=4))
        wpool2 = ctx.enter_context(tc.tile_pool(name="wpool2", bufs=3))
        wpool3 = ctx.enter_context(tc.tile_pool(name="wpool3", bufs=3))
        ptpool = ctx.enter_context(tc.tile_pool(name="ptpool", bufs=4))
        rows2 = ctx.enter_context(tc.tile_pool(name="rows2", bufs=3))
        bcast = ctx.enter_context(tc.tile_pool(name="bcast", bufs=4))
        bcast2 = ctx.enter_context(tc.tile_pool(name="bcast2", bufs=2))
        rbpool = ctx.enter_context(tc.tile_pool(name="rbpool", bufs=2))
        outpool = ctx.enter_context(tc.tile_pool(name="outpool", bufs=2))
        xqpool = ctx.enter_context(tc.tile_pool(name="xqpool", bufs=2))
        mpool = ctx.enter_context(tc.tile_pool(name="mpool", bufs=3))
        mstream = ctx.enter_context(tc.tile_pool(name="mstream", bufs=2))
        mdram = ctx.enter_context(tc.tile_pool(name="mdram", bufs=1, space="DRAM"))

        # ---------- constants / biases ----------
        ones_col = persist.tile([P, 1], F32R, tag="ones_col")
        nc.sync.dma_start(ones_col, ones_d.rearrange("(p o) -> p o", o=1))
        ones_k1 = persist.tile([1, P], BF16, tag="ones_k1")
        nc.gpsimd.dma_start(ones_k1, ones_d.rearrange("(o p) -> o p", o=1))
        epst = persist.tile([1, 1], F32, tag="eps")
        nc.vector.memset(epst, EPS)
        qkb_sb = persist.tile([P, 16], F32, tag="qkb")
        nc.sync.dma_start(qkb_sb, qkvb_qk.rearrange("(o p) -> p o", p=P))
        fc1b_sb = persist.tile([P, 32], F32, tag="fc1b")
        nc.sync.dma_start(fc1b_sb, fc1b.rearrange("(o p) -> p o", p=P))
        fc2b_sb = persist.tile([P, 8], F32, tag="fc2b")
        nc.sync.dma_start(fc2b_sb, fc2b.rearrange("(o p) -> p o", p=P))
        vb_row = persist.tile([1, C], BF16, tag="vb")
        nc.sync.dma_start(vb_row, qkvb_v.ap())
        ones_k1f = persist.tile([1, P], F32R, tag="ones_k1f")
        nc.sync.dma_start(ones_k1f, ones_d.rearrange("(o p) -> o p", o=1))

        # mask (bf16, [k, q] as [128, kb, q])
        mask_sb = persist.tile([P, KB, NQ], BF16, tag="mask")
        mT = maskT.rearrange("(kb p) q -> p kb q", p=P)
        nc.sync.dma_start(mask_sb, mT)

        # ---------- phases 1+2: LN1, pipelined per token-half ----------
        # hT is split into two tiles (one per 512-token half) so the QKV
        # matmuls for half 0 can start while half 1 is still normalizing.
        xTr = xT.rearrange("(ct p) n -> p ct n", p=P)
        hTh = []
        psA_ctx = ExitStack()
        psA = psA_ctx.enter_context(tc.tile_pool(name="psA", bufs=2, space="PSUM"))
        for nh in range(2):
            sl = slice(nh * 512, (nh + 1) * 512)
            ps_s1 = psA.tile([1, 512], F32, tag="s", name=f"ps_s1_{nh}")
            ps_s2 = psA.tile([1, 512], F32, tag="s2", name=f"ps_s2_{nh}")
            for t in range(CT // 2):
                xc = xstream.tile([P, 2, 512], F32R, tag="xc", name=f"xc_{nh}_{t}")
                nc.sync.dma_start(xc, xTr[:, 2 * t:2 * t + 2, sl])
                for i in range(2):
                    ct = 2 * t + i
                    sq = sqpool.tile([P, 512], F32R, tag="sq", name=f"sq_{nh}_{ct}")
                    nc.scalar.activation(sq, xc[:, i, :].bitcast(F32), AF.Square)
                    nc.tensor.matmul(ps_s1, ones_col, xc[:, i, :],
                                     start=(ct == 0), stop=(ct == CT - 1))
                    nc.tensor.matmul(ps_s2, ones_col, sq,
                                     start=(ct == 0), stop=(ct == CT - 1))
            mu_row = rows2.tile([1, 512], F32, tag="r512ln", name=f"mu_{nh}")
            nc.vector.tensor_scalar_mul(mu_row, ps_s1, 1.0 / C)
            e2_row = rows2.tile([1, 512], F32, tag="r512ln", name=f"e2_{nh}")
            nc.vector.tensor_scalar_mul(e2_row, ps_s2, 1.0 / C)
            tmp_row = rows2.tile([1, 512], F32, tag="r512ln", name=f"tmp_{nh}")
            nc.vector.tensor_tensor(tmp_row, mu_row, mu_row, ALU.mult)
            nc.vector.tensor_tensor(e2_row, e2_row, tmp_row, ALU.subtract)
            nc.scalar.activation(e2_row, e2_row, AF.Sqrt, bias=epst[:, :])
            nc.vector.reciprocal(e2_row, e2_row)
            mu_bc = bcast.tile([P, 512], F32, tag="b512", name=f"mu_b_{nh}")
            nc.gpsimd.partition_broadcast(mu_bc[:, :], mu_row[:, :])
            rs_bc = bcast.tile([P, 512], F32, tag="b512", name=f"rs_b_{nh}")
            nc.gpsimd.partition_broadcast(rs_bc[:, :], e2_row[:, :])
            hT = hpool.tile([P, CT, 512], BF16, tag=f"h{nh}")
            tmpf = hpool.tile([P, 512], F32, tag=f"htmp{nh}")
            for t in range(CT // 2):
                xc = xstream.tile([P, 2, 512], F32R, tag="xc", name=f"xc2_{nh}_{t}")
                nc.sync.dma_start(xc, xTr[:, 2 * t:2 * t + 2, sl])
                for i in range(2):
                    ct = 2 * t + i
                    nc.vector.tensor_tensor(tmpf, xc[:, i, :].bitcast(F32), mu_bc,
                                            ALU.subtract)
                    nc.vector.tensor_tensor(hT[:, ct, :], tmpf, rs_bc, ALU.mult)
            hTh.append(hT)
        psA_ctx.close()

        if stop_after == 'ln1':
            return nc
        # ---------- phase 3: q^T, k^T (channel-major, f32r matmul) ----------
        # Host passes xT ROLLED per core: the query half is always tokens
        # [0:512] (= hTh[0]); K/V cover both halves with mask columns matched.
        qT = persist.tile([P, CT, NQ], BF16, tag="qT")
        kT = persist.tile([P, CT, NK], BF16, tag="kT")
        wT = qkv_wT.rearrange("(ct p) m -> p ct m", p=P)
        psB_ctx = ExitStack()
        psB = psB_ctx.enter_context(tc.tile_pool(name="psB", bufs=4, space="PSUM"))
        for wg in range(4):          # 4 groups of 4 chan-blocks (q,q,k,k)
            is_q = wg < 2
            nhs = [0] if is_q else [0, 1]
            for nh in nhs:
                tsl = slice(nh * 512, (nh + 1) * 512)
                pss = [psB.tile([P, 512], F32, tag="mm", name=f"ps_qk{wg}_{nh}_{j}")
                       for j in range(4)]
                for t in range(CT // 2):
                    wt = wpool.tile([P, 2, 512], BF16, tag="w")
                    nc.sync.dma_start(wt, wT[:, 2 * t:2 * t + 2, wg * 512:(wg + 1) * 512])
                    for i in range(2):
                        ct = 2 * t + i
                        for j in range(4):
                            nc.tensor.matmul(pss[j], wt[:, i, j * P:(j + 1) * P],
                                             hTh[nh][:, ct, :],
                                             start=(ct == 0), stop=(ct == CT - 1))
                for j in range(4):
                    mb = wg * 4 + j
                    bias = qkb_sb[:, mb:mb + 1]
                    if is_q:
                        nc.scalar.activation(qT[:, mb, :], pss[j], AF.Identity, bias=bias)
                    else:
                        nc.scalar.activation(kT[:, mb - 8, tsl], pss[j], AF.Identity,
                                             bias=bias)

        if stop_after == 'qk':
            psB_ctx.close()
            return nc
        # ---------- phase 4: v token-major + ones column ----------
        # V_aug: [128 tok, kb, head, 65] (65th col = 1.0 for denominator)
        V_aug = persist.tile([P, KB, H, 65], BF16, tag="vaug")
        nc.vector.memset(V_aug[:, :, :, 64:65], 1.0)
        psV_ctx = ExitStack()
        psV = psV_ctx.enter_context(tc.tile_pool(name="psV", bufs=4, space="PSUM"))
        for nh in range(4):
            wvs = []
            for t in range(CT // 2):
                wv = vwpool.tile([P, 2, 256], BF16, tag="vw", name=f"vw_{nh}_{t}")
                nc.sync.dma_start(wv, wT[:, 2 * t:2 * t + 2,
                                         2048 + nh * 256: 2048 + (nh + 1) * 256])
                wvs.append(wv)
            for tb in range(KB):
                hsrc = hTh[tb // 4][:, :, (tb % 4) * P:(tb % 4 + 1) * P]
                psv = psV.tile([P, 256], F32, tag="mmv", name=f"psv_{nh}_{tb}")
                for ct in range(CT):
                    nc.tensor.matmul(psv, hsrc[:, ct, :],
                                     wvs[ct // 2][:, ct % 2, :],
                                     start=(ct == 0), stop=False)
                nc.tensor.matmul(psv, ones_k1,
                                 vb_row[:, nh * 256:(nh + 1) * 256],
                                 start=False, stop=True)
                nc.scalar.activation(
                    V_aug[:, tb, nh * 4:(nh + 1) * 4, 0:64],
                    psv.rearrange("p (h d) -> p h d", d=64),
                    AF.Copy)
        psV_ctx.close()
        psB_ctx.close()

        if stop_after == 'v':
            return nc
        # ---------- phase 5: attention per head ----------
        # S psums in [128, 2, 512] groups -> 1024-wide exp and mask ops.
        # Mask-multiply alternates DVE / GPSIMD to split the elementwise load.
        psS_ctx = ExitStack()
        psS = psS_ctx.enter_context(tc.tile_pool(name="psS", bufs=2, space="PSUM"))
        psO = psS_ctx.enter_context(tc.tile_pool(name="psO", bufs=2, space="PSUM"))
        oT = persist.tile([P, CT, NQ], BF16, tag="oT")
        for h in range(H):
            j, base = h // 2, (h % 2) * 64
            po = psO.tile([65, NQ], F32, tag="o")
            for g in range(4):
                psg = psS.tile([P, 2, 512], F32, tag="sg", name=f"psg_{h}_{g}")
                for i in range(2):
                    kb = 2 * g + i
                    nc.tensor.matmul(psg[:, i, :],
                                     kT[base:base + 64, j, kb * P:(kb + 1) * P],
                                     qT[base:base + 64, j, :], start=True, stop=True)
                PT = ptpool.tile([P, 2, NQ], BF16, tag="pt", name=f"pt_{h}_{g}")
                nc.scalar.activation(PT, psg, AF.Exp)
                nc.vector.tensor_tensor(PT, PT, mask_sb[:, 2 * g:2 * g + 2, :], ALU.mult)
                for i in range(2):
                    kb = 2 * g + i
                    nc.tensor.matmul(po, V_aug[:, kb, h, :], PT[:, i, :],
                                     start=(kb == 0), stop=(kb == KB - 1))
            drow = rows2.tile([1, NQ], F32, tag="r512")
            nc.vector.tensor_scalar_add(drow, po[64:65, :], 1.0)
            nc.vector.reciprocal(drow, drow)
            rb = rbpool.tile([64, NQ], F32, tag="rb")
            nc.gpsimd.partition_broadcast(rb[:, :], drow[:, :])
            nc.vector.tensor_tensor(oT[base:base + 64, j, :], po[0:64, :], rb, ALU.mult)
        psS_ctx.close()

        if stop_after == 'attn':
            return nc
        # ---------- phase 6: proj + residual -> x2T ----------
        psB2_ctx = ExitStack()
        psB2 = psB2_ctx.enter_context(tc.tile_pool(name="psB2", bufs=4, space="PSUM"))
        x2T = persist.tile([P, CT, NQ], F32R, tag="x2T")
        pTr = proj_wT.rearrange("(ct p) m -> p ct m", p=P)
        for ob in range(CT):
            wp = wpool3.tile([P, CT, P], BF16, tag="wp")
            nc.sync.dma_start(wp, pTr[:, :, ob * P:(ob + 1) * P])
            psp = psB2.tile([P, 512], F32, tag="mm")
            for ct in range(CT):
                nc.tensor.matmul(psp, wp[:, ct], oT[:, ct, :],
                                 start=(ct == 0), stop=(ct == CT - 1))
            xq = xqpool.tile([P, NQ], F32, tag="xq")
            nc.sync.dma_start(xq, xTqb[ob * P:(ob + 1) * P, :])
            nc.vector.tensor_tensor(x2T[:, ob, :], psp, xq, ALU.add)

        if stop_after == 'proj':
            psB2_ctx.close()
            return nc
        # ---------- phase 7: LN2 ----------
        psC_ctx = ExitStack()
        psC = psC_ctx.enter_context(tc.tile_pool(name="psC", bufs=2, space="PSUM"))
        ps_t1 = psC.tile([1, NQ], F32, tag="s2")
        ps_t2 = psC.tile([1, NQ], F32, tag="s2")
        for ob in range(CT):
            sq2 = sqpool.tile([P, 512], F32R, tag="sq")
            nc.scalar.activation(sq2, x2T[:, ob, :].bitcast(F32), AF.Square)
            nc.tensor.matmul(ps_t1, ones_col, x2T[:, ob, :],
                             start=(ob == 0), stop=(ob == CT - 1))
            nc.tensor.matmul(ps_t2, ones_col, sq2,
                             start=(ob == 0), stop=(ob == CT - 1))
        mu2 = rows2.tile([1, NQ], F32R, tag="r512ln")
        nc.vector.tensor_scalar_mul(mu2, ps_t1, 1.0 / C)
        e22 = rows2.tile([1, NQ], F32R, tag="r512ln")
        nc.vector.tensor_scalar_mul(e22, ps_t2, 1.0 / C)
        tmp2 = rows2.tile([1, NQ], F32R, tag="r512ln")
        nc.vector.tensor_tensor(tmp2, mu2.bitcast(F32), mu2.bitcast(F32), ALU.mult)
        nc.vector.tensor_tensor(e22, e22.bitcast(F32), tmp2.bitcast(F32), ALU.subtract)
        nc.scalar.activation(e22, e22.bitcast(F32), AF.Sqrt, bias=epst[:, :])
        with nc.allow_low_precision(reason="f32r rsig rounding ~1e-4 is fine"):
            nc.vector.reciprocal(e22, e22.bitcast(F32))
        rs2 = e22
        mu2b = psC.tile([P, NQ], F32, tag="bc2")
        nc.tensor.matmul(mu2b, ones_k1f, mu2, start=True, stop=True)
        rs2b = psC.tile([P, NQ], F32, tag="bc2")
        nc.tensor.matmul(rs2b, ones_k1f, rs2, start=True, stop=True)
        h2T = hpool.tile([P, CT, NQ], BF16, tag="h0")
        h2tmp = hpool.tile([P, 512], F32, tag="htmp0")
        for ob in range(CT):
            nc.vector.tensor_tensor(h2tmp, x2T[:, ob, :].bitcast(F32), mu2b,
                                    ALU.subtract)
            nc.vector.tensor_tensor(h2T[:, ob, :], h2tmp, rs2b, ALU.mult)
        psC_ctx.close()

        if stop_after == 'ln2':
            psB2_ctx.close()
            return nc
        # ---------- phase 8a: fc1 + gelu -> m (bf16, spilled to DRAM) ----------
        m_dram = mdram.tile([32, P, NQ], BF16)
        w1T = fc1_wT.rearrange("(ct p) m -> p ct m", p=P)
        for hg in range(8):
            pss = [psB2.tile([P, 512], F32, tag="mm", name=f"ps_fc1_{hg}_{j}")
                   for j in range(4)]
            w1 = wpool.tile([P, CT, 512], BF16, tag="w1")
            nc.sync.dma_start(w1, w1T[:, :, hg * 512:(hg + 1) * 512])
            for ct in range(CT):
                for j in range(4):
                    nc.tensor.matmul(pss[j], w1[:, ct, j * P:(j + 1) * P],
                                     h2T[:, ct, :],
                                     start=(ct == 0), stop=(ct == CT - 1))
            for j in range(4):
                hb = hg * 4 + j
                mo = mpool.tile([P, NQ], BF16, tag="mo")
                nc.scalar.activation(mo, pss[j], AF.Gelu, bias=fc1b_sb[:, hb:hb + 1])
                nc.sync.dma_start(m_dram[hb], mo)
        psB2_ctx.close()

        if stop_after == 'fc1':
            return nc
        # ---------- phase 8b: fc2 + bias + residual -> out ----------
        psD_ctx = ExitStack()
        psD = psD_ctx.enter_context(tc.tile_pool(name="psD", bufs=8, space="PSUM"))
        ps_oc = [psD.tile([P, 512], F32, tag="fc2", name=f"ps_fc2_{ob}")
                 for ob in range(8)]
        w2T = fc2_wT.rearrange("(ht p) m -> p ht m", p=P)
        for tp in range(16):
            w2 = wpool2.tile([P, 2, C], BF16, tag="w2")
            nc.sync.dma_start(w2, w2T[:, 2 * tp:2 * tp + 2, :])
            mi = mstream.tile([P, 2, NQ], BF16, tag="mi")
            nc.sync.dma_start(mi, m_dram[2 * tp:2 * tp + 2].rearrange("h p q -> p h q"))
            for i in range(2):
                ht = 2 * tp + i
                for ob in range(CT):
                    nc.tensor.matmul(ps_oc[ob], w2[:, i, ob * P:(ob + 1) * P],
                                     mi[:, i, :],
                                     start=(ht == 0), stop=(ht == 31))
        for ob in range(CT):
            ot = outpool.tile([P, NQ], F32, tag="out")
            nc.vector.tensor_scalar(ot, ps_oc[ob], fc2b_sb[:, ob:ob + 1], None, ALU.add)
            nc.vector.tensor_tensor(ot, ot, x2T[:, ob, :].bitcast(F32), ALU.add)
            nc.sync.dma_start(outT[ob * P:(ob + 1) * P, :], ot)
        psD_ctx.close()

    return nc


# ---------------------------------------------------------------------------
# Host side: shard, run, gather
# ---------------------------------------------------------------------------
_RUNNER = None


class _Runner:
    """Minimal SPMD executor via bass2jax custom call (axon PJRT path)."""

    def __init__(self, nc, n_cores):
        import jax
        from jax.sharding import Mesh, PartitionSpec
        from jax.experimental.shard_map import shard_map
        from concourse.bass2jax import (_bass_exec_p, install_neuronx_cc_hook,
                                        partition_id_tensor)
        install_neuronx_cc_hook()
        self.jax = jax
        self.nc = nc
        self.n_cores = n_cores
        partition_name = nc.partition_id_tensor.name if nc.partition_id_tensor else None
        in_names, out_names, out_avals, zero_outs = [], [], [], []
        for alloc in nc.m.functions[0].allocations:
            if not isinstance(alloc, mybir.MemoryLocationSet):
                continue
            name = alloc.memorylocations[0].name
            if alloc.kind == "ExternalInput":
                if name != partition_name:
                    in_names.append(name)
            elif alloc.kind == "ExternalOutput":
                shape = tuple(alloc.tensor_shape)
                dtype = mybir.dt.np(alloc.dtype)
                out_names.append(name)
                out_avals.append(jax.core.ShapedArray(shape, dtype))
                zero_outs.append(np.zeros(shape, dtype))
        self.in_names, self.out_names = in_names, out_names
        self._out_avals, self._zero_outs = out_avals, zero_outs
        n_params = len(in_names)
        all_in = in_names + out_names + ([partition_name] if partition_name else [])

        def _body(*args):
            operands = list(args)
            if partition_name is not None:
                operands.append(partition_id_tensor())
            return tuple(_bass_exec_p.bind(
                *operands, out_avals=tuple(out_avals), in_names=tuple(all_in),
                out_names=tuple(out_names), lowering_input_output_aliases=(),
                sim_require_finite=True, sim_require_nnan=True, nc=nc))

        devices = jax.devices()[:n_cores]
        mesh = Mesh(np.asarray(devices), ("core",))
        nspec = n_params + len(out_names)
        self._fn = jax.jit(
            shard_map(_body, mesh=mesh, in_specs=(PartitionSpec("core"),) * nspec,
                      out_specs=(PartitionSpec("core"),) * len(out_names),
                      check_rep=False),
            keep_unused=True)

    def run(self, in_maps):
        n = self.n_cores
        per_core = [[np.ascontiguousarray(m[k]) for k in self.in_names] for m in in_maps]
        args = [np.concatenate([per_core[c][i] for c in range(n)], axis=0)
                for i in range(len(self.in_names))]
        args += [np.zeros((n * z.shape[0], *z.shape[1:]), z.dtype) for z in self._zero_outs]
        outs = self._fn(*args)
        self.jax.block_until_ready(outs)
        return [
            {name: np.asarray(outs[i]).reshape(n, *self._out_avals[i].shape)[c]
             for i, name in enumerate(self.out_names)}
            for c in range(n)
        ]


def _get_runner():
    global _RUNNER
    if _RUNNER is None:
        _RUNNER = _Runner(build_nc(), 8)
    return _RUNNER


def kernel(x, mask, ln1_g, ln1_b, qkv_w, qkv_b, proj_w, proj_b,
           ln2_g, ln2_b, fc1_w, fc1_b, fc2_w, fc2_b):
    x = np.asarray(x, np.float32)
    mask = np.asarray(mask, bool)
    ln1_g = np.asarray(ln1_g, np.float32); ln1_b = np.asarray(ln1_b, np.float32)
    qkv_w = np.asarray(qkv_w, np.float32); qkv_b = np.asarray(qkv_b, np.float32)
    proj_w = np.asarray(proj_w, np.float32); proj_b = np.asarray(proj_b, np.float32)
    ln2_g = np.asarray(ln2_g, np.float32); ln2_b = np.asarray(ln2_b, np.float32)
    fc1_w = np.asarray(fc1_w, np.float32); fc1_b = np.asarray(fc1_b, np.float32)
    fc2_w = np.asarray(fc2_w, np.float32); fc2_b = np.asarray(fc2_b, np.float32)

    # fold LN1 gain/bias + softmax scale into qkv weights/bias
    Wq = qkv_w * ln1_g[None, :]
    qb = qkv_w @ ln1_b + qkv_b
    Wq[:C] *= SCALE
    qb = qb.copy(); qb[:C] *= SCALE
    qkv_wT = np.ascontiguousarray(Wq.T).astype(ml_dtypes.bfloat16)
    # fold LN2 gain/bias into fc1
    W1 = fc1_w * ln2_g[None, :]
    fb1 = fc1_w @ ln2_b + fc1_b
    fc1_wT = np.ascontiguousarray(W1.T).astype(ml_dtypes.bfloat16)
    proj_wT = np.ascontiguousarray(proj_w.T).astype(ml_dtypes.bfloat16)
    fc2_wT = np.ascontiguousarray(fc2_w.T).astype(ml_dtypes.bfloat16)
    qkvb_v = np.ascontiguousarray(qb[2 * C:].reshape(1, C)).astype(ml_dtypes.bfloat16)
    maskmul = (~mask).astype(ml_dtypes.bfloat16)

    in_maps = []
    for core in range(8):
        b, s = core // 2, core % 2
        # roll x so this core's query half is always tokens [0:512]
        xb = x[b]                         # [N, C]
        xroll = np.roll(xb, -s * NQ, axis=0)
        xT_c = np.ascontiguousarray(xroll.T)               # [C, NK]
        xTqb_c = np.ascontiguousarray(xb[s * NQ:(s + 1) * NQ].T) + proj_b[:, None]
        # mask rows q = this core's queries; key order must match rolled order
        mrow = maskmul[b, s * NQ:(s + 1) * NQ]             # [NQ, N] keys orig order
        mroll = np.roll(mrow, -s * NQ, axis=1)             # keys in rolled order
        maskT_c = np.ascontiguousarray(mroll.T)            # [NK, NQ]
        in_maps.append({
            "ones_d": np.ones(P, np.float32),
            "xT": xT_c.astype(np.float32),
            "xTqb": xTqb_c.astype(np.float32),
            "maskT": maskT_c,
            "qkv_wT": qkv_wT, "qkvb_qk": qb[:2 * C].copy(), "qkvb_v": qkvb_v,
            "proj_wT": proj_wT,
            "fc1_wT": fc1_wT, "fc1b": fb1.copy(),
            "fc2_wT": fc2_wT, "fc2b": fc2_b.copy(),
        })

    results = _get_runner().run(in_maps)
    out = np.empty((B, N, C), np.float32)
    for core in range(8):
        b, s = core // 2, core % 2
        out[b, s * NQ:(s + 1) * NQ, :] = results[core]["outT"].T
    return out



# revision 17
# speedup vs baseline: 1.1285x; 1.1285x over previous
"""Trainium2 Bass kernel for a pre-norm transformer block (B=4, N=1024, C=1024,
16 heads, MLP hidden 4096), SPMD across 8 NeuronCores.

Sharding: core = (b, s) with b = batch element (4), s = query-half (2).
Each core computes the block for 512 query tokens of one batch element:
  - LN1 + K/V projections over the full 1024-token sequence of its batch
    element (duplicated within the batch pair - avoids all collectives),
  - Q projection + attention + proj + residual + LN2 + MLP for its 512 rows.

Everything on-device is channel-major (channels on partitions, tokens on the
free dim) so no on-device transposes are needed; the host passes x.T and
pre-transposed weights. LayerNorm gains/biases are folded into the following
matmul weights/biases on the host; the softmax scale is folded into the Q
weights. Softmax uses no max-subtraction (logits are O(1) by construction),
which makes the softmax1 "+1" denominator term exact and free; the
denominator itself comes from a ones-column appended to V inside the P@V
matmul.

Schedule notes (v2): x is DMA'd once into a resident f32 tile (freed before
the MLP so its SBUF is reused for the fc1 activations, which never touch
DRAM); the mask load rides the Pool/SWDGE queue so it doesn't delay the x
stream; QKV runs in 8-PSUM-bank rounds so K weights load once; proj/fc2
weights are pre-tiled on the host so every DMA descriptor is >=2KB; fc2 runs
output-block-outer so the bias+residual+store epilogue of block i overlaps
the matmuls of block i+1.

Precision: main matmuls in float32r (full PE rate, ~1e-4 matmul rel err);
attention S/P/O and the MLP hidden activations in bf16; everything else fp32.
"""

from contextlib import ExitStack

import numpy as np
import ml_dtypes

import concourse.bass as bass
import concourse.mybir as mybir
import concourse.tile as tile
from concourse import bacc

P = 128
B, N, C = 4, 1024, 1024
H, D = 16, 64
HD = 4 * C
NQ = N // 2          # query tokens per core
NK = N               # key/value tokens per core
CT = C // P          # 8 channel tiles
HT = HD // P         # 32 hidden tiles
KB = NK // P         # 8 key-token tiles
SCALE = D ** (-0.5)
EPS = 1e-5

F32 = mybir.dt.float32
F32R = mybir.dt.float32r
BF16 = mybir.dt.bfloat16
AF = mybir.ActivationFunctionType
ALU = mybir.AluOpType


def build_nc(stop_after=None):
    nc = _build_body(stop_after)
    nc.compile()
    return nc


def _build_body(stop_after=None):
    nc = bacc.Bacc("TRN2", target_bir_lowering=False, debug=False, num_devices=8)

    # ---- DRAM I/O (per core) ----
    xT = nc.dram_tensor("xT", [C, NK], F32R, kind="ExternalInput")
    ones_d = nc.dram_tensor("ones_d", [P], F32R, kind="ExternalInput")
    maskT = nc.dram_tensor("maskT", [NK, NQ], BF16, kind="ExternalInput")  # 1-mask, transposed
    qkv_wT = nc.dram_tensor("qkv_wT", [C, 3 * C], BF16, kind="ExternalInput")
    qkvb_qk = nc.dram_tensor("qkvb_qk", [2 * C], F32, kind="ExternalInput")
    qkvb_v = nc.dram_tensor("qkvb_v", [1, C], BF16, kind="ExternalInput")
    proj_wTt = nc.dram_tensor("proj_wTt", [CT, P, CT, P], BF16, kind="ExternalInput")
    projb = nc.dram_tensor("projb", [C], F32, kind="ExternalInput")
    fc1_wT = nc.dram_tensor("fc1_wT", [C, HD], BF16, kind="ExternalInput")
    fc1b = nc.dram_tensor("fc1b", [HD], F32, kind="ExternalInput")
    fc2_wTt = nc.dram_tensor("fc2_wTt", [CT, P, HT, P], BF16, kind="ExternalInput")
    fc2b = nc.dram_tensor("fc2b", [C], F32, kind="ExternalInput")
    outT = nc.dram_tensor("outT", [C, NQ], F32, kind="ExternalOutput")

    with tile.TileContext(nc) as tc, ExitStack() as ctx:
        persist = ctx.enter_context(tc.tile_pool(name="persist", bufs=1))
        hpool = ctx.enter_context(tc.tile_pool(name="hpool", bufs=1))
        sqpool = ctx.enter_context(tc.tile_pool(name="sqpool", bufs=2))
        wpool = ctx.enter_context(tc.tile_pool(name="wpool", bufs=4))
        vwpool = ctx.enter_context(tc.tile_pool(name="vwpool", bufs=8))
        wpool3 = ctx.enter_context(tc.tile_pool(name="wpool3", bufs=2))
        ptpool = ctx.enter_context(tc.tile_pool(name="ptpool", bufs=3))
        rows2 = ctx.enter_context(tc.tile_pool(name="rows2", bufs=3))
        bcast = ctx.enter_context(tc.tile_pool(name="bcast", bufs=2))
        rbpool = ctx.enter_context(tc.tile_pool(name="rbpool", bufs=2))
        outpool = ctx.enter_context(tc.tile_pool(name="outpool", bufs=2))

        # scoped: x (f32, 32KB/partition) lives until the proj residual; the
        # attention tensors live until proj. Both release before the MLP pools
        # allocate, so SBUF peaks stay under the per-partition budget.
        xfull_ctx = ExitStack()
        xpool = xfull_ctx.enter_context(tc.tile_pool(name="xpool", bufs=1))
        attn_ctx = ExitStack()
        apool = attn_ctx.enter_context(tc.tile_pool(name="apool", bufs=1))

        # ---------- startup DMAs: ones (for LN sums), then x, then consts ----
        ones_col = persist.tile([P, 1], F32R, tag="ones_col")
        nc.sync.dma_start(ones_col, ones_d.rearrange("(p o) -> p o", o=1))

        xf = xpool.tile([P, CT, NK], F32R, tag="xf")
        xTr = xT.rearrange("(ct p) n -> p ct n", p=P)
        # first two cts as single-ct chunks so the very first sum/square can
        # start ~0.7us earlier; the rest as 2-ct chunks
        nc.sync.dma_start(xf[:, 0:1, 0:NQ], xTr[:, 0:1, 0:NQ])
        nc.sync.dma_start(xf[:, 1:2, 0:NQ], xTr[:, 1:2, 0:NQ])
        for t in range(1, CT // 2):
            nc.sync.dma_start(xf[:, 2 * t:2 * t + 2, 0:NQ], xTr[:, 2 * t:2 * t + 2, 0:NQ])
        for t in range(CT // 2):
            nc.sync.dma_start(xf[:, 2 * t:2 * t + 2, NQ:NK], xTr[:, 2 * t:2 * t + 2, NQ:NK])

        epst = persist.tile([1, 1], F32, tag="eps")
        nc.vector.memset(epst, EPS)
        # dummy sqrt: pulls the sqrt act-table load off LN1's critical path
        # (the set also contains square/identity/copy, so nothing else loads
        # a table until attention's exp).
        scr11 = persist.tile([1, 1], F32, tag="scr")
        nc.scalar.activation(scr11, epst, AF.Sqrt)
        qkb_sb = persist.tile([P, 16], F32, tag="qkb")
        nc.sync.dma_start(qkb_sb, qkvb_qk.rearrange("(o p) -> p o", p=P))
        fc1b_sb = persist.tile([P, HT], F32, tag="fc1b")
        nc.sync.dma_start(fc1b_sb, fc1b.rearrange("(o p) -> p o", p=P))
        fc2b_sb = persist.tile([P, CT], F32, tag="fc2b")
        nc.sync.dma_start(fc2b_sb, fc2b.rearrange("(o p) -> p o", p=P))
        projb_sb = persist.tile([P, CT], F32, tag="projb")
        nc.sync.dma_start(projb_sb, projb.rearrange("(o p) -> p o", p=P))
        vb_row = persist.tile([1, C], BF16, tag="vb")
        nc.sync.dma_start(vb_row, qkvb_v.ap())
        ones_k1 = persist.tile([1, P], BF16, tag="ones_k1")
        nc.gpsimd.dma_start(ones_k1, ones_d.rearrange("(o p) -> o p", o=1))

        # mask (bf16, [k, q] as [128, kb, q]) on the Pool/SWDGE queue so it
        # never delays the x / weight streams on the HWDGE path.
        mask_sb = apool.tile([P, KB, NQ], BF16, tag="mask")
        nc.gpsimd.dma_start(mask_sb, maskT.rearrange("(kb p) q -> p kb q", p=P))

        # ---------- phase 1+2: LN1 (sums -> stats -> normalize per half) ----
        hTh = []
        psA_ctx = ExitStack()
        psA = psA_ctx.enter_context(tc.tile_pool(name="psA", bufs=2, space="PSUM"))
        stats = []
        for nh in range(2):
            sl = slice(nh * NQ, (nh + 1) * NQ)
            ps_s1 = psA.tile([1, NQ], F32, tag="s", name=f"ps_s1_{nh}")
            ps_s2 = psA.tile([1, NQ], F32, tag="s2", name=f"ps_s2_{nh}")
            for ct in range(CT):
                sq = sqpool.tile([P, NQ], F32R, tag="sq", name=f"sq_{nh}_{ct}")
                xc = xf[:, ct, sl]
                if ct % 2 == 0:
                    nc.scalar.activation(sq, xc.bitcast(F32), AF.Square)
                else:
                    nc.vector.tensor_tensor(sq, xc.bitcast(F32), xc.bitcast(F32),
                                            ALU.mult)
                nc.tensor.matmul(ps_s1, ones_col, xc,
                                 start=(ct == 0), stop=(ct == CT - 1))
                nc.tensor.matmul(ps_s2, ones_col, sq,
                                 start=(ct == 0), stop=(ct == CT - 1))
            stats.append((ps_s1, ps_s2))
        for nh in range(2):
            sl = slice(nh * NQ, (nh + 1) * NQ)
            ps_s1, ps_s2 = stats[nh]
            mu_row = rows2.tile([1, NQ], F32, tag="r512ln", name=f"mu_{nh}")
            nc.vector.tensor_scalar_mul(mu_row, ps_s1, 1.0 / C)
            e2_row = rows2.tile([1, NQ], F32, tag="r512ln", name=f"e2_{nh}")
            nc.vector.tensor_scalar_mul(e2_row, ps_s2, 1.0 / C)
            tmp_row = rows2.tile([1, NQ], F32, tag="r512ln", name=f"tmp_{nh}")
            nc.vector.tensor_tensor(tmp_row, mu_row, mu_row, ALU.mult)
            nc.vector.tensor_tensor(e2_row, e2_row, tmp_row, ALU.subtract)
            nc.scalar.activation(e2_row, e2_row, AF.Sqrt, bias=epst[:, :])
            nc.vector.reciprocal(e2_row, e2_row)
            mu_bc = bcast.tile([P, NQ], F32, tag="b512", name=f"mu_b_{nh}")
            nc.gpsimd.partition_broadcast(mu_bc[:, :], mu_row[:, :])
            rs_bc = bcast.tile([P, NQ], F32, tag="b512", name=f"rs_b_{nh}")
            nc.gpsimd.partition_broadcast(rs_bc[:, :], e2_row[:, :])
            hT = hpool.tile([P, CT, NQ], BF16, tag=f"h{nh}")
            tmpf = hpool.tile([P, NQ], F32, tag=f"htmp{nh}")
            for ct in range(CT):
                nc.vector.tensor_tensor(tmpf, xf[:, ct, sl].bitcast(F32), mu_bc,
                                        ALU.subtract)
                nc.vector.tensor_tensor(hT[:, ct, :], tmpf, rs_bc, ALU.mult)
            hTh.append(hT)
        psA_ctx.close()

        if stop_after == 'ln1':
            attn_ctx.close()
            xfull_ctx.close()
            return nc
        # ---------- phase 3: q^T, k^T in 8-PSUM-bank rounds ----------
        # Host passes xT ROLLED per core: the query half is always tokens
        # [0:512] (= hTh[0]); K/V cover both halves with mask columns matched.
        qT = apool.tile([P, CT, NQ], BF16, tag="qT")
        kT = apool.tile([P, CT, NK], BF16, tag="kT")
        wT = qkv_wT.rearrange("(ct p) m -> p ct m", p=P)
        # psV allocated BEFORE psB so the V psums own banks that the Q/K bias
        # drains never touch - V matmuls start the moment K matmuls end.
        psV_ctx = ExitStack()
        psV = psV_ctx.enter_context(tc.tile_pool(name="psV", bufs=4, space="PSUM"))
        psB_ctx = ExitStack()
        psB = psB_ctx.enter_context(tc.tile_pool(name="psB", bufs=4, space="PSUM"))
        # Q: two 4-psum rounds sharing one set of weight tiles (loaded once)
        wq_tiles = []
        for t in range(CT // 2):
            wt = wpool.tile([P, 2, 2 * NQ], BF16, tag="w", name=f"wq_{t}")
            nc.sync.dma_start(wt, wT[:, 2 * t:2 * t + 2, 0:C])
            wq_tiles.append(wt)
        for qr in range(2):
            pss = [psB.tile([P, NQ], F32, tag="mm", name=f"ps_q{qr}_{j}")
                   for j in range(4)]
            for ct in range(CT):
                for j in range(4):
                    jj = qr * 4 + j
                    nc.tensor.matmul(pss[j], wq_tiles[ct // 2][:, ct % 2,
                                                              jj * P:(jj + 1) * P],
                                     hTh[0][:, ct, :],
                                     start=(ct == 0), stop=(ct == CT - 1))
            for j in range(4):
                jj = qr * 4 + j
                nc.scalar.activation(qT[:, jj, :], pss[j], AF.Identity,
                                     bias=qkb_sb[:, jj:jj + 1])
        # K: two weight groups x two 4-psum rounds; biases split ACT/DVE
        for kr in range(2):
            wk_tiles = []
            for t in range(CT // 2):
                wt = wpool.tile([P, 2, NQ], BF16, tag="w", name=f"wk{kr}_{t}")
                nc.sync.dma_start(wt, wT[:, 2 * t:2 * t + 2,
                                         C + kr * 512:C + (kr + 1) * 512])
                wk_tiles.append(wt)
            for half in range(2):
                pss = [psB.tile([P, NQ], F32, tag="mm", name=f"ps_k{kr}_{half}_{x}")
                       for x in range(4)]
                for ct in range(CT):
                    for j2 in range(2):
                        for nh in range(2):
                            j = half * 2 + j2
                            nc.tensor.matmul(
                                pss[2 * j2 + nh],
                                wk_tiles[ct // 2][:, ct % 2, j * P:(j + 1) * P],
                                hTh[nh][:, ct, :],
                                start=(ct == 0), stop=(ct == CT - 1))
                for j2 in range(2):
                    j = half * 2 + j2
                    mb = 8 + kr * 4 + j
                    nc.scalar.activation(
                        kT[:, mb - 8, 0:NQ], pss[2 * j2],
                        AF.Identity, bias=qkb_sb[:, mb:mb + 1])
                    nc.vector.tensor_scalar(
                        kT[:, mb - 8, NQ:NK], pss[2 * j2 + 1],
                        qkb_sb[:, mb:mb + 1], None, ALU.add)
        psB_ctx.close()

        if stop_after == 'qk':
            psV_ctx.close()
            attn_ctx.close()
            xfull_ctx.close()
            return nc
        # ---------- phase 4: v token-major + ones column ----------
        # V_aug: [128 tok, kb, head, 65] (65th col = 1.0 for denominator)
        V_aug = apool.tile([P, KB, H, 65], BF16, tag="vaug")
        nc.vector.memset(V_aug[:, :, :, 64:65], 1.0)
        for nh in range(4):
            wvs = []
            for t in range(CT // 2):
                wv = vwpool.tile([P, 2, 256], BF16, tag="vw", name=f"vw_{nh}_{t}")
                nc.sync.dma_start(wv, wT[:, 2 * t:2 * t + 2,
                                         2048 + nh * 256: 2048 + (nh + 1) * 256])
                wvs.append(wv)
            for tb in range(KB):
                hsrc = hTh[tb // 4][:, :, (tb % 4) * P:(tb % 4 + 1) * P]
                psv = psV.tile([P, 256], F32, tag="mmv", name=f"psv_{nh}_{tb}")
                for ct in range(CT):
                    nc.tensor.matmul(psv, hsrc[:, ct, :],
                                     wvs[ct // 2][:, ct % 2, :],
                                     start=(ct == 0), stop=False)
                nc.tensor.matmul(psv, ones_k1,
                                 vb_row[:, nh * 256:(nh + 1) * 256],
                                 start=False, stop=True)
                # drain copies alternate ACT/DVE so psum banks free 2x faster
                if tb % 2 == 0:
                    nc.scalar.activation(
                        V_aug[:, tb, nh * 4:(nh + 1) * 4, 0:64],
                        psv.rearrange("p (h d) -> p h d", d=64),
                        AF.Copy)
                else:
                    nc.vector.tensor_copy(
                        V_aug[:, tb, nh * 4:(nh + 1) * 4, 0:64],
                        psv.rearrange("p (h d) -> p h d", d=64))
        psV_ctx.close()

        if stop_after == 'v':
            attn_ctx.close()
            xfull_ctx.close()
            return nc
        # ---------- phase 5: attention per head ----------
        # S psums in [128, 2, 512] groups -> 1024-wide exp and mask ops.
        psS_ctx = ExitStack()
        psS = psS_ctx.enter_context(tc.tile_pool(name="psS", bufs=2, space="PSUM"))
        psO = psS_ctx.enter_context(tc.tile_pool(name="psO", bufs=2, space="PSUM"))
        oT = apool.tile([P, CT, NQ], BF16, tag="oT")
        for h in range(H):
            j, base = h // 2, (h % 2) * 64
            po = psO.tile([65, NQ], F32, tag="o")
            for g in range(4):
                psg = psS.tile([P, 2, NQ], F32, tag="sg", name=f"psg_{h}_{g}")
                for i in range(2):
                    kb = 2 * g + i
                    nc.tensor.matmul(psg[:, i, :],
                                     kT[base:base + 64, j, kb * P:(kb + 1) * P],
                                     qT[base:base + 64, j, :], start=True, stop=True)
                PT = ptpool.tile([P, 2, NQ], BF16, tag="pt", name=f"pt_{h}_{g}")
                nc.scalar.activation(PT, psg, AF.Exp)
                nc.vector.tensor_tensor(PT, PT, mask_sb[:, 2 * g:2 * g + 2, :], ALU.mult)
                for i in range(2):
                    kb = 2 * g + i
                    nc.tensor.matmul(po, V_aug[:, kb, h, :], PT[:, i, :],
                                     start=(kb == 0), stop=(kb == KB - 1))
            drow = rows2.tile([1, NQ], F32, tag="r512", bufs=2)
            nc.vector.tensor_scalar_add(drow, po[64:65, :], 1.0)
            nc.vector.reciprocal(drow, drow)
            rb = rbpool.tile([64, NQ], F32, tag="rb")
            nc.gpsimd.partition_broadcast(rb[:, :], drow[:, :])
            nc.vector.tensor_tensor(oT[base:base + 64, j, :], po[0:64, :], rb, ALU.mult)
        psS_ctx.close()
        # dummy sqrt: reloads the sqrt act-table now (ACT idle, PE busy with
        # proj) so LN2's real sqrt doesn't pay the 1.3us switch from exp.
        nc.scalar.activation(scr11, epst, AF.Sqrt)

        if stop_after == 'attn':
            attn_ctx.close()
            xfull_ctx.close()
            return nc
        # ---------- phase 6: proj + residual (+proj bias) -> x2T ----------
        # LN2's channel sums are interleaved one ob behind the proj matmuls so
        # only the stats tail (~3us) remains after the last proj block.
        psC_ctx = ExitStack()
        psC = psC_ctx.enter_context(tc.tile_pool(name="psC", bufs=2, space="PSUM"))
        psB2_ctx = ExitStack()
        psB2 = psB2_ctx.enter_context(tc.tile_pool(name="psB2", bufs=4, space="PSUM"))
        ps_t1 = psC.tile([1, NQ], F32, tag="s2")
        ps_t2 = psC.tile([1, NQ], F32, tag="s2")
        x2T = persist.tile([P, CT, NQ], F32R, tag="x2T")

        def ln2_sums(ob):
            # squares on ACT (idle during proj), sums on PE; DVE only does the
            # residual add so it stays ahead producing x2T
            sq2 = sqpool.tile([P, NQ], F32R, tag="sq", name=f"sq2_{ob}")
            nc.scalar.activation(sq2, x2T[:, ob, :].bitcast(F32), AF.Square)
            nc.tensor.matmul(ps_t1, ones_col, x2T[:, ob, :],
                             start=(ob == 0), stop=(ob == CT - 1))
            nc.tensor.matmul(ps_t2, ones_col, sq2,
                             start=(ob == 0), stop=(ob == CT - 1))

        for ob in range(CT):
            wp = wpool3.tile([P, CT, P], BF16, tag="wp")
            nc.sync.dma_start(wp, proj_wTt[ob])
            psp = psB2.tile([P, NQ], F32, tag="mm")
            for ct in range(CT):
                nc.tensor.matmul(psp, wp[:, ct], oT[:, ct, :],
                                 start=(ct == 0), stop=(ct == CT - 1))
            nc.scalar.activation(x2T[:, ob, :], psp, AF.Identity,
                                 bias=projb_sb[:, ob:ob + 1])
            nc.vector.tensor_tensor(x2T[:, ob, :], x2T[:, ob, :].bitcast(F32),
                                    xf[:, ob, 0:NQ].bitcast(F32), ALU.add)
            if ob > 0:
                ln2_sums(ob - 1)
        ln2_sums(CT - 1)

        attn_ctx.close()
        xfull_ctx.close()
        psB2_ctx.close()

        if stop_after == 'proj':
            psC_ctx.close()
            return nc
        # ---------- phase 7: LN2 stats + normalize ----------
        # prefetch the first two fc1 weight groups while the stats chain runs
        w1T = fc1_wT.rearrange("(ct p) m -> p ct m", p=P)
        w1_tiles = {}
        for hg in range(2):
            w1 = wpool.tile([P, CT, NQ], BF16, tag="w1", bufs=2, name=f"w1_{hg}")
            nc.sync.dma_start(w1, w1T[:, :, hg * 512:(hg + 1) * 512])
            w1_tiles[hg] = w1
        mu2 = rows2.tile([1, NQ], F32R, tag="r512ln")
        nc.vector.tensor_scalar_mul(mu2, ps_t1, 1.0 / C)
        mu2b = bcast.tile([P, NQ], F32, tag="b512")
        nc.gpsimd.partition_broadcast(mu2b[:, :], mu2.bitcast(F32)[:, :])
        e22 = rows2.tile([1, NQ], F32R, tag="r512ln")
        nc.vector.tensor_scalar_mul(e22, ps_t2, 1.0 / C)
        tmp2 = rows2.tile([1, NQ], F32R, tag="r512ln")
        nc.vector.tensor_tensor(tmp2, mu2.bitcast(F32), mu2.bitcast(F32), ALU.mult)
        nc.vector.tensor_tensor(e22, e22.bitcast(F32), tmp2.bitcast(F32), ALU.subtract)
        nc.scalar.activation(e22, e22.bitcast(F32), AF.Sqrt, bias=epst[:, :])
        with nc.allow_low_precision(reason="f32r rsig rounding ~1e-4 is fine"):
            nc.vector.reciprocal(e22, e22.bitcast(F32))
        rs2b = bcast.tile([P, NQ], F32, tag="b512")
        nc.gpsimd.partition_broadcast(rs2b[:, :], e22.bitcast(F32)[:, :])
        psC_ctx.close()
        h2T = hpool.tile([P, CT, NQ], BF16, tag="h0")
        h2tmp = hpool.tile([P, NQ], F32, tag="htmp0")
        for ob in range(CT):
            nc.vector.tensor_tensor(h2tmp, x2T[:, ob, :].bitcast(F32), mu2b,
                                    ALU.subtract)
            nc.vector.tensor_tensor(h2T[:, ob, :], h2tmp, rs2b, ALU.mult)

        if stop_after == 'ln2':
            return nc
        # ---------- phase 8a: fc1 + gelu -> m (bf16, resident in SBUF) ----------
        mlp_ctx = ExitStack()
        mpool = mlp_ctx.enter_context(tc.tile_pool(name="mpool", bufs=1))
        wpool2 = mlp_ctx.enter_context(tc.tile_pool(name="wpool2", bufs=2))
        psF_ctx = ExitStack()
        psF = psF_ctx.enter_context(tc.tile_pool(name="psF", bufs=8, space="PSUM"))
        m_sb = mpool.tile([P, HT, NQ], BF16, tag="m")
        for hg in range(8):
            pss = [psF.tile([P, NQ], F32, tag="mm", name=f"ps_fc1_{hg}_{j}")
                   for j in range(4)]
            if hg in w1_tiles:
                w1 = w1_tiles[hg]
            else:
                w1 = wpool.tile([P, CT, NQ], BF16, tag="w1", bufs=2, name=f"w1_{hg}")
                nc.sync.dma_start(w1, w1T[:, :, hg * 512:(hg + 1) * 512])
            for ct in range(CT):
                for j in range(4):
                    nc.tensor.matmul(pss[j], w1[:, ct, j * P:(j + 1) * P],
                                     h2T[:, ct, :],
                                     start=(ct == 0), stop=(ct == CT - 1))
            for j in range(4):
                hb = hg * 4 + j
                nc.scalar.activation(m_sb[:, hb, :], pss[j], AF.Gelu,
                                     bias=fc1b_sb[:, hb:hb + 1])
        psF_ctx.close()

        if stop_after == 'fc1':
            mlp_ctx.close()
            return nc
        # ---------- phase 8b: fc2 + bias + residual -> out (ob-outer) ----------
        psD_ctx = ExitStack()
        psD = psD_ctx.enter_context(tc.tile_pool(name="psD", bufs=2, space="PSUM"))
        for ob in range(CT):
            w2 = wpool2.tile([P, HT, P], BF16, tag="w2")
            nc.sync.dma_start(w2, fc2_wTt[ob])
            ps = psD.tile([P, NQ], F32, tag="fc2", name=f"ps_fc2_{ob}")
            for ht in range(HT):
                nc.tensor.matmul(ps, w2[:, ht, :], m_sb[:, ht, :],
                                 start=(ht == 0), stop=(ht == HT - 1))
            ot = outpool.tile([P, NQ], F32, tag="out")
            if ob < CT - 1:
                nc.vector.tensor_scalar(ot, ps, fc2b_sb[:, ob:ob + 1], None, ALU.add)
                nc.vector.tensor_tensor(ot, ot, x2T[:, ob, :].bitcast(F32), ALU.add)
                nc.sync.dma_start(outT[ob * P:(ob + 1) * P, :], ot)
            else:
                # last block: pipeline the epilogue in halves to shorten the
                # kernel tail (store of half A overlaps math of half B)
                for hh in range(2):
                    hs = slice(hh * 256, (hh + 1) * 256)
                    nc.vector.tensor_scalar(ot[:, hs], ps[:, hs],
                                            fc2b_sb[:, ob:ob + 1], None, ALU.add)
                    nc.vector.tensor_tensor(ot[:, hs], ot[:, hs],
                                            x2T[:, ob, hs].bitcast(F32), ALU.add)
                    nc.sync.dma_start(outT[ob * P:(ob + 1) * P, hs], ot[:, hs])
        psD_ctx.close()
        mlp_ctx.close()

    return nc


# ---------------------------------------------------------------------------
# Host side: shard, run, gather
# ---------------------------------------------------------------------------
_RUNNER = None


class _Runner:
    """Minimal SPMD executor via bass2jax custom call (axon PJRT path)."""

    def __init__(self, nc, n_cores):
        import jax
        from jax.sharding import Mesh, PartitionSpec
        from jax.experimental.shard_map import shard_map
        from concourse.bass2jax import (_bass_exec_p, install_neuronx_cc_hook,
                                        partition_id_tensor)
        install_neuronx_cc_hook()
        self.jax = jax
        self.nc = nc
        self.n_cores = n_cores
        partition_name = nc.partition_id_tensor.name if nc.partition_id_tensor else None
        in_names, out_names, out_avals, zero_outs = [], [], [], []
        for alloc in nc.m.functions[0].allocations:
            if not isinstance(alloc, mybir.MemoryLocationSet):
                continue
            name = alloc.memorylocations[0].name
            if alloc.kind == "ExternalInput":
                if name != partition_name:
                    in_names.append(name)
            elif alloc.kind == "ExternalOutput":
                shape = tuple(alloc.tensor_shape)
                dtype = mybir.dt.np(alloc.dtype)
                out_names.append(name)
                out_avals.append(jax.core.ShapedArray(shape, dtype))
                zero_outs.append(np.zeros(shape, dtype))
        self.in_names, self.out_names = in_names, out_names
        self._out_avals, self._zero_outs = out_avals, zero_outs
        n_params = len(in_names)
        all_in = in_names + out_names + ([partition_name] if partition_name else [])

        def _body(*args):
            operands = list(args)
            if partition_name is not None:
                operands.append(partition_id_tensor())
            return tuple(_bass_exec_p.bind(
                *operands, out_avals=tuple(out_avals), in_names=tuple(all_in),
                out_names=tuple(out_names), lowering_input_output_aliases=(),
                sim_require_finite=True, sim_require_nnan=True, nc=nc))

        devices = jax.devices()[:n_cores]
        mesh = Mesh(np.asarray(devices), ("core",))
        nspec = n_params + len(out_names)
        self._fn = jax.jit(
            shard_map(_body, mesh=mesh, in_specs=(PartitionSpec("core"),) * nspec,
                      out_specs=(PartitionSpec("core"),) * len(out_names),
                      check_rep=False),
            keep_unused=True)

    def run(self, in_maps):
        n = self.n_cores
        per_core = [[np.ascontiguousarray(m[k]) for k in self.in_names] for m in in_maps]
        args = [np.concatenate([per_core[c][i] for c in range(n)], axis=0)
                for i in range(len(self.in_names))]
        args += [np.zeros((n * z.shape[0], *z.shape[1:]), z.dtype) for z in self._zero_outs]
        outs = self._fn(*args)
        self.jax.block_until_ready(outs)
        return [
            {name: np.asarray(outs[i]).reshape(n, *self._out_avals[i].shape)[c]
             for i, name in enumerate(self.out_names)}
            for c in range(n)
        ]


def _get_runner():
    global _RUNNER
    if _RUNNER is None:
        _RUNNER = _Runner(build_nc(), 8)
    return _RUNNER


def kernel(x, mask, ln1_g, ln1_b, qkv_w, qkv_b, proj_w, proj_b,
           ln2_g, ln2_b, fc1_w, fc1_b, fc2_w, fc2_b):
    x = np.asarray(x, np.float32)
    mask = np.asarray(mask, bool)
    ln1_g = np.asarray(ln1_g, np.float32); ln1_b = np.asarray(ln1_b, np.float32)
    qkv_w = np.asarray(qkv_w, np.float32); qkv_b = np.asarray(qkv_b, np.float32)
    proj_w = np.asarray(proj_w, np.float32); proj_b = np.asarray(proj_b, np.float32)
    ln2_g = np.asarray(ln2_g, np.float32); ln2_b = np.asarray(ln2_b, np.float32)
    fc1_w = np.asarray(fc1_w, np.float32); fc1_b = np.asarray(fc1_b, np.float32)
    fc2_w = np.asarray(fc2_w, np.float32); fc2_b = np.asarray(fc2_b, np.float32)

    # fold LN1 gain/bias + softmax scale into qkv weights/bias
    Wq = qkv_w * ln1_g[None, :]
    qb = qkv_w @ ln1_b + qkv_b
    Wq[:C] *= SCALE
    qb = qb.copy(); qb[:C] *= SCALE
    qkv_wT = np.ascontiguousarray(Wq.T).astype(ml_dtypes.bfloat16)
    # fold LN2 gain/bias into fc1
    W1 = fc1_w * ln2_g[None, :]
    fb1 = fc1_w @ ln2_b + fc1_b
    fc1_wT = np.ascontiguousarray(W1.T).astype(ml_dtypes.bfloat16)
    # proj/fc2 weights pre-tiled as [out_blk, partition, in_blk, out_col] so
    # each per-block DMA is one fully-contiguous 2KB+/partition transfer.
    proj_wTt = np.ascontiguousarray(
        proj_w.T.reshape(CT, P, CT, P).transpose(2, 1, 0, 3)
    ).astype(ml_dtypes.bfloat16)
    fc2_wTt = np.ascontiguousarray(
        fc2_w.T.reshape(HT, P, CT, P).transpose(2, 1, 0, 3)
    ).astype(ml_dtypes.bfloat16)
    qkvb_v = np.ascontiguousarray(qb[2 * C:].reshape(1, C)).astype(ml_dtypes.bfloat16)
    maskmul = (~mask).astype(ml_dtypes.bfloat16)

    in_maps = []
    for core in range(8):
        b, s = core // 2, core % 2
        # roll x so this core's query half is always tokens [0:512]
        xb = x[b]                         # [N, C]
        xroll = np.roll(xb, -s * NQ, axis=0)
        xT_c = np.ascontiguousarray(xroll.T)               # [C, NK]
        # mask rows q = this core's queries; key order must match rolled order
        mrow = maskmul[b, s * NQ:(s + 1) * NQ]             # [NQ, N] keys orig order
        mroll = np.roll(mrow, -s * NQ, axis=1)             # keys in rolled order
        maskT_c = np.ascontiguousarray(mroll.T)            # [NK, NQ]
        in_maps.append({
            "ones_d": np.ones(P, np.float32),
            "xT": xT_c.astype(np.float32),
            "maskT": maskT_c,
            "qkv_wT": qkv_wT, "qkvb_qk": qb[:2 * C].copy(), "qkvb_v": qkvb_v,
            "proj_wTt": proj_wTt, "projb": proj_b.copy(),
            "fc1_wT": fc1_wT, "fc1b": fb1.copy(),
            "fc2_wTt": fc2_wTt, "fc2b": fc2_b.copy(),
        })

    results = _get_runner().run(in_maps)
    out = np.empty((B, N, C), np.float32)
    for core in range(8):
        b, s = core // 2, core % 2
        out[b, s * NQ:(s + 1) * NQ, :] = results[core]["outT"].T
    return out


# revision 23
# speedup vs baseline: 1.1396x; 1.0098x over previous
"""Trainium2 Bass kernel for a pre-norm transformer block (B=4, N=1024, C=1024,
16 heads, MLP hidden 4096), SPMD across 8 NeuronCores.

Sharding: core = (b, s) with b = batch element (4), s = query-half (2).
Each core computes the block for 512 query tokens of one batch element:
  - LN1 + K/V projections over the full 1024-token sequence of its batch
    element (duplicated within the batch pair - avoids all collectives),
  - Q projection + attention + proj + residual + LN2 + MLP for its 512 rows.

Everything on-device is channel-major (channels on partitions, tokens on the
free dim) so no on-device transposes are needed; the host passes x.T and
pre-transposed weights. LayerNorm gains/biases are folded into the following
matmul weights/biases on the host; the softmax scale is folded into the Q
weights. Softmax uses no max-subtraction (logits are O(1) by construction),
which makes the softmax1 "+1" denominator term exact and free; the
denominator itself comes from a ones-column appended to V inside the P@V
matmul.

Schedule notes (v2): x is DMA'd once into a resident f32 tile (freed before
the MLP so its SBUF is reused for the fc1 activations, which never touch
DRAM); the mask load rides the Pool/SWDGE queue so it doesn't delay the x
stream; QKV runs in 8-PSUM-bank rounds so K weights load once; proj/fc2
weights are pre-tiled on the host so every DMA descriptor is >=2KB; fc2 runs
output-block-outer so the bias+residual+store epilogue of block i overlaps
the matmuls of block i+1.

Precision: main matmuls in float32r (full PE rate, ~1e-4 matmul rel err);
attention S/P/O and the MLP hidden activations in bf16; everything else fp32.
"""

from contextlib import ExitStack

import numpy as np
import ml_dtypes

import concourse.bass as bass
import concourse.mybir as mybir
import concourse.tile as tile
from concourse import bacc

P = 128
B, N, C = 4, 1024, 1024
H, D = 16, 64
HD = 4 * C
NQ = N // 2          # query tokens per core
NK = N               # key/value tokens per core
CT = C // P          # 8 channel tiles
HT = HD // P         # 32 hidden tiles
KB = NK // P         # 8 key-token tiles
SCALE = D ** (-0.5)
EPS = 1e-5

F32 = mybir.dt.float32
F32R = mybir.dt.float32r
BF16 = mybir.dt.bfloat16
AF = mybir.ActivationFunctionType
ALU = mybir.AluOpType


def build_nc(stop_after=None):
    nc = _build_body(stop_after)
    nc.compile()
    return nc


def _build_body(stop_after=None):
    nc = bacc.Bacc("TRN2", target_bir_lowering=False, debug=False, num_devices=8)

    # ---- DRAM I/O (per core) ----
    xT = nc.dram_tensor("xT", [C, NK], F32R, kind="ExternalInput")
    ones_d = nc.dram_tensor("ones_d", [P], F32R, kind="ExternalInput")
    maskT = nc.dram_tensor("maskT", [NK, NQ], BF16, kind="ExternalInput")  # 1-mask, transposed
    qkv_wT = nc.dram_tensor("qkv_wT", [C, 3 * C], BF16, kind="ExternalInput")
    qkvb_qk = nc.dram_tensor("qkvb_qk", [2 * C], F32, kind="ExternalInput")
    qkvb_v = nc.dram_tensor("qkvb_v", [1, C], BF16, kind="ExternalInput")
    proj_wTt = nc.dram_tensor("proj_wTt", [CT, P, CT, P], BF16, kind="ExternalInput")
    projb = nc.dram_tensor("projb", [C], F32, kind="ExternalInput")
    fc1_wT = nc.dram_tensor("fc1_wT", [C, HD], BF16, kind="ExternalInput")
    fc1b = nc.dram_tensor("fc1b", [HD], F32, kind="ExternalInput")
    fc2_wTt = nc.dram_tensor("fc2_wTt", [CT, P, HT, P], BF16, kind="ExternalInput")
    fc2b = nc.dram_tensor("fc2b", [C], F32, kind="ExternalInput")
    outT = nc.dram_tensor("outT", [C, NQ], F32, kind="ExternalOutput")

    with tile.TileContext(nc) as tc, ExitStack() as ctx:
        persist = ctx.enter_context(tc.tile_pool(name="persist", bufs=1))
        hpool = ctx.enter_context(tc.tile_pool(name="hpool", bufs=1))
        sqpool = ctx.enter_context(tc.tile_pool(name="sqpool", bufs=2))
        wpool = ctx.enter_context(tc.tile_pool(name="wpool", bufs=4))
        vwpool = ctx.enter_context(tc.tile_pool(name="vwpool", bufs=8))
        wpool3 = ctx.enter_context(tc.tile_pool(name="wpool3", bufs=2))
        ptpool = ctx.enter_context(tc.tile_pool(name="ptpool", bufs=3))
        rows2 = ctx.enter_context(tc.tile_pool(name="rows2", bufs=3))
        bcast = ctx.enter_context(tc.tile_pool(name="bcast", bufs=2))
        rbpool = ctx.enter_context(tc.tile_pool(name="rbpool", bufs=2))
        outpool = ctx.enter_context(tc.tile_pool(name="outpool", bufs=2))

        # scoped: x (f32, 32KB/partition) lives until the proj residual; the
        # attention tensors live until proj. Both release before the MLP pools
        # allocate, so SBUF peaks stay under the per-partition budget.
        xfull_ctx = ExitStack()
        xpool = xfull_ctx.enter_context(tc.tile_pool(name="xpool", bufs=1))
        attn_ctx = ExitStack()
        apool = attn_ctx.enter_context(tc.tile_pool(name="apool", bufs=1))

        # ---------- startup DMAs: ones (for LN sums), then x, then consts ----
        ones_col = persist.tile([P, 1], F32R, tag="ones_col")
        nc.sync.dma_start(ones_col, ones_d.rearrange("(p o) -> p o", o=1))

        xf = xpool.tile([P, CT, NK], F32R, tag="xf")
        xTr = xT.rearrange("(ct p) n -> p ct n", p=P)
        # first two cts as single-ct chunks so the very first sum/square can
        # start ~0.7us earlier; the rest as 2-ct chunks
        nc.sync.dma_start(xf[:, 0:1, 0:NQ], xTr[:, 0:1, 0:NQ])
        nc.sync.dma_start(xf[:, 1:2, 0:NQ], xTr[:, 1:2, 0:NQ])
        for t in range(1, CT // 2):
            nc.sync.dma_start(xf[:, 2 * t:2 * t + 2, 0:NQ], xTr[:, 2 * t:2 * t + 2, 0:NQ])
        for t in range(CT // 2):
            nc.sync.dma_start(xf[:, 2 * t:2 * t + 2, NQ:NK], xTr[:, 2 * t:2 * t + 2, NQ:NK])

        epst = persist.tile([1, 1], F32, tag="eps")
        nc.vector.memset(epst, EPS)
        # dummy sqrt: pulls the sqrt act-table load off LN1's critical path
        # (the set also contains square/identity/copy, so nothing else loads
        # a table until attention's exp).
        scr11 = persist.tile([1, 1], F32, tag="scr")
        nc.scalar.activation(scr11, epst, AF.Sqrt)
        qkb_sb = persist.tile([P, 16], F32, tag="qkb")
        nc.sync.dma_start(qkb_sb, qkvb_qk.rearrange("(o p) -> p o", p=P))
        fc1b_sb = persist.tile([P, HT], F32, tag="fc1b")
        nc.sync.dma_start(fc1b_sb, fc1b.rearrange("(o p) -> p o", p=P))
        fc2b_sb = persist.tile([P, CT], F32, tag="fc2b")
        nc.sync.dma_start(fc2b_sb, fc2b.rearrange("(o p) -> p o", p=P))
        projb_sb = persist.tile([P, CT], F32, tag="projb")
        nc.sync.dma_start(projb_sb, projb.rearrange("(o p) -> p o", p=P))
        vb_row = persist.tile([1, C], BF16, tag="vb")
        nc.sync.dma_start(vb_row, qkvb_v.ap())
        ones_k1 = persist.tile([1, P], BF16, tag="ones_k1")
        nc.gpsimd.dma_start(ones_k1, ones_d.rearrange("(o p) -> o p", o=1))
        # consts for folding softmax1's "+1" denominator term into the P@V
        # psum as a rank-1 matmul (e65 x ones_q adds 1.0 to the denom row)
        e65 = persist.tile([1, 65], BF16, tag="e65")
        nc.vector.memset(e65[:, 0:64], 0.0)
        nc.vector.memset(e65[:, 64:65], 1.0)
        ones_q = persist.tile([1, NQ], BF16, tag="ones_q")
        nc.vector.memset(ones_q, 1.0)

        # mask (bf16, [k, q] as [128, kb, q]) on the Pool/SWDGE queue so it
        # never delays the x / weight streams on the HWDGE path.
        mask_sb = apool.tile([P, KB, NQ], BF16, tag="mask")
        nc.gpsimd.dma_start(mask_sb, maskT.rearrange("(kb p) q -> p kb q", p=P))

        # ---------- phase 1+2: LN1 (sums -> stats -> normalize per half) ----
        hTh = []
        psA_ctx = ExitStack()
        psA = psA_ctx.enter_context(tc.tile_pool(name="psA", bufs=2, space="PSUM"))
        stats = []
        for nh in range(2):
            sl = slice(nh * NQ, (nh + 1) * NQ)
            ps_s1 = psA.tile([1, NQ], F32, tag="s", name=f"ps_s1_{nh}")
            ps_s2 = psA.tile([1, NQ], F32, tag="s2", name=f"ps_s2_{nh}")
            for ct in range(CT):
                sq = sqpool.tile([P, NQ], F32R, tag="sq", name=f"sq_{nh}_{ct}")
                xc = xf[:, ct, sl]
                if ct % 2 == 0:
                    nc.scalar.activation(sq, xc.bitcast(F32), AF.Square)
                else:
                    nc.vector.tensor_tensor(sq, xc.bitcast(F32), xc.bitcast(F32),
                                            ALU.mult)
                nc.tensor.matmul(ps_s1, ones_col, xc,
                                 start=(ct == 0), stop=(ct == CT - 1))
                nc.tensor.matmul(ps_s2, ones_col, sq,
                                 start=(ct == 0), stop=(ct == CT - 1))
            stats.append((ps_s1, ps_s2))
        for nh in range(2):
            sl = slice(nh * NQ, (nh + 1) * NQ)
            ps_s1, ps_s2 = stats[nh]
            mu_row = rows2.tile([1, NQ], F32, tag="r512ln", name=f"mu_{nh}")
            nc.vector.tensor_scalar_mul(mu_row, ps_s1, 1.0 / C)
            e2_row = rows2.tile([1, NQ], F32, tag="r512ln", name=f"e2_{nh}")
            nc.vector.tensor_scalar_mul(e2_row, ps_s2, 1.0 / C)
            tmp_row = rows2.tile([1, NQ], F32, tag="r512ln", name=f"tmp_{nh}")
            nc.vector.tensor_tensor(tmp_row, mu_row, mu_row, ALU.mult)
            nc.vector.tensor_tensor(e2_row, e2_row, tmp_row, ALU.subtract)
            nc.scalar.activation(e2_row, e2_row, AF.Sqrt, bias=epst[:, :])
            nc.vector.reciprocal(e2_row, e2_row)
            mu_bc = bcast.tile([P, NQ], F32, tag="b512", name=f"mu_b_{nh}")
            nc.gpsimd.partition_broadcast(mu_bc[:, :], mu_row[:, :])
            rs_bc = bcast.tile([P, NQ], F32, tag="b512", name=f"rs_b_{nh}")
            nc.gpsimd.partition_broadcast(rs_bc[:, :], e2_row[:, :])
            hT = hpool.tile([P, CT, NQ], BF16, tag=f"h{nh}")
            tmpf = hpool.tile([P, NQ], F32, tag=f"htmp{nh}")
            for ct in range(CT):
                nc.vector.tensor_tensor(tmpf, xf[:, ct, sl].bitcast(F32), mu_bc,
                                        ALU.subtract)
                nc.vector.tensor_tensor(hT[:, ct, :], tmpf, rs_bc, ALU.mult)
            hTh.append(hT)
        psA_ctx.close()

        if stop_after == 'ln1':
            attn_ctx.close()
            xfull_ctx.close()
            return nc
        # ---------- phase 3: q^T, k^T in 8-PSUM-bank rounds ----------
        # Host passes xT ROLLED per core: the query half is always tokens
        # [0:512] (= hTh[0]); K/V cover both halves with mask columns matched.
        qT = apool.tile([P, CT, NQ], BF16, tag="qT")
        kT = apool.tile([P, CT, NK], BF16, tag="kT")
        wT = qkv_wT.rearrange("(ct p) m -> p ct m", p=P)
        # psV allocated BEFORE psB so the V psums own banks that the Q/K bias
        # drains never touch - V matmuls start the moment K matmuls end.
        psV_ctx = ExitStack()
        psV = psV_ctx.enter_context(tc.tile_pool(name="psV", bufs=4, space="PSUM"))
        psB_ctx = ExitStack()
        psB = psB_ctx.enter_context(tc.tile_pool(name="psB", bufs=4, space="PSUM"))
        # Q: two 4-psum rounds sharing one set of weight tiles (loaded once)
        wq_tiles = []
        for t in range(CT // 2):
            wt = wpool.tile([P, 2, 2 * NQ], BF16, tag="w", name=f"wq_{t}")
            nc.sync.dma_start(wt, wT[:, 2 * t:2 * t + 2, 0:C])
            wq_tiles.append(wt)
        for qr in range(2):
            pss = [psB.tile([P, NQ], F32, tag="mm", name=f"ps_q{qr}_{j}")
                   for j in range(4)]
            for ct in range(CT):
                for j in range(4):
                    jj = qr * 4 + j
                    nc.tensor.matmul(pss[j], wq_tiles[ct // 2][:, ct % 2,
                                                              jj * P:(jj + 1) * P],
                                     hTh[0][:, ct, :],
                                     start=(ct == 0), stop=(ct == CT - 1))
            for j in range(4):
                jj = qr * 4 + j
                nc.scalar.activation(qT[:, jj, :], pss[j], AF.Identity,
                                     bias=qkb_sb[:, jj:jj + 1])
        # K: two weight groups x two 4-psum rounds; biases split ACT/DVE
        for kr in range(2):
            wk_tiles = []
            for t in range(CT // 2):
                wt = wpool.tile([P, 2, NQ], BF16, tag="w", name=f"wk{kr}_{t}")
                nc.sync.dma_start(wt, wT[:, 2 * t:2 * t + 2,
                                         C + kr * 512:C + (kr + 1) * 512])
                wk_tiles.append(wt)
            for half in range(2):
                pss = [psB.tile([P, NQ], F32, tag="mm", name=f"ps_k{kr}_{half}_{x}")
                       for x in range(4)]
                for ct in range(CT):
                    for j2 in range(2):
                        for nh in range(2):
                            j = half * 2 + j2
                            nc.tensor.matmul(
                                pss[2 * j2 + nh],
                                wk_tiles[ct // 2][:, ct % 2, j * P:(j + 1) * P],
                                hTh[nh][:, ct, :],
                                start=(ct == 0), stop=(ct == CT - 1))
                for j2 in range(2):
                    j = half * 2 + j2
                    mb = 8 + kr * 4 + j
                    nc.scalar.activation(
                        kT[:, mb - 8, 0:NQ], pss[2 * j2],
                        AF.Identity, bias=qkb_sb[:, mb:mb + 1])
                    nc.vector.tensor_scalar(
                        kT[:, mb - 8, NQ:NK], pss[2 * j2 + 1],
                        qkb_sb[:, mb:mb + 1], None, ALU.add)
        psB_ctx.close()

        if stop_after == 'qk':
            psV_ctx.close()
            attn_ctx.close()
            xfull_ctx.close()
            return nc
        # ---------- phase 4: v token-major + ones column ----------
        # V_aug: [128 tok, kb, head, 65] (65th col = 1.0 for denominator)
        V_aug = apool.tile([P, KB, H, 65], BF16, tag="vaug")
        nc.vector.memset(V_aug[:, :, :, 64:65], 1.0)
        for nh in range(4):
            wvs = []
            for t in range(CT // 2):
                wv = vwpool.tile([P, 2, 256], BF16, tag="vw", name=f"vw_{nh}_{t}")
                nc.sync.dma_start(wv, wT[:, 2 * t:2 * t + 2,
                                         2048 + nh * 256: 2048 + (nh + 1) * 256])
                wvs.append(wv)
            for tb in range(KB):
                hsrc = hTh[tb // 4][:, :, (tb % 4) * P:(tb % 4 + 1) * P]
                psv = psV.tile([P, 256], F32, tag="mmv", name=f"psv_{nh}_{tb}")
                for ct in range(CT):
                    nc.tensor.matmul(psv, hsrc[:, ct, :],
                                     wvs[ct // 2][:, ct % 2, :],
                                     start=(ct == 0), stop=False)
                nc.tensor.matmul(psv, ones_k1,
                                 vb_row[:, nh * 256:(nh + 1) * 256],
                                 start=False, stop=True)
                # drain copies alternate ACT/DVE so psum banks free 2x faster
                if tb % 2 == 0:
                    nc.scalar.activation(
                        V_aug[:, tb, nh * 4:(nh + 1) * 4, 0:64],
                        psv.rearrange("p (h d) -> p h d", d=64),
                        AF.Copy)
                else:
                    nc.vector.tensor_copy(
                        V_aug[:, tb, nh * 4:(nh + 1) * 4, 0:64],
                        psv.rearrange("p (h d) -> p h d", d=64))
        psV_ctx.close()

        if stop_after == 'v':
            attn_ctx.close()
            xfull_ctx.close()
            return nc
        # ---------- phase 5: attention per head ----------
        # S psums in [128, 2, 512] groups -> 1024-wide exp and mask ops.
        psS_ctx = ExitStack()
        psS = psS_ctx.enter_context(tc.tile_pool(name="psS", bufs=3, space="PSUM"))
        psO = psS_ctx.enter_context(tc.tile_pool(name="psO", bufs=2, space="PSUM"))
        oT = apool.tile([P, CT, NQ], BF16, tag="oT")
        for h in range(H):
            j, base = h // 2, (h % 2) * 64
            po = psO.tile([65, NQ], F32, tag="o")
            for g in range(4):
                psg = psS.tile([P, 2, NQ], F32, tag="sg", name=f"psg_{h}_{g}")
                for i in range(2):
                    kb = 2 * g + i
                    nc.tensor.matmul(psg[:, i, :],
                                     kT[base:base + 64, j, kb * P:(kb + 1) * P],
                                     qT[base:base + 64, j, :], start=True, stop=True)
                PT = ptpool.tile([P, 2, NQ], BF16, tag="pt", name=f"pt_{h}_{g}")
                nc.scalar.activation(PT, psg, AF.Exp)
                nc.vector.tensor_tensor(PT, PT, mask_sb[:, 2 * g:2 * g + 2, :], ALU.mult)
                for i in range(2):
                    kb = 2 * g + i
                    nc.tensor.matmul(po, V_aug[:, kb, h, :], PT[:, i, :],
                                     start=(kb == 0), stop=False)
            nc.tensor.matmul(po, e65, ones_q, start=False, stop=True)
            drow = rows2.tile([1, NQ], F32, tag="r512", bufs=2)
            nc.vector.reciprocal(drow, po[64:65, :])
            rb = rbpool.tile([64, NQ], F32, tag="rb")
            nc.gpsimd.partition_broadcast(rb[:, :], drow[:, :])
            nc.vector.tensor_tensor(oT[base:base + 64, j, :], po[0:64, :], rb, ALU.mult)
        # dummy sqrt emitted inside the attention scope (before the pool
        # release barrier): reloads the sqrt act-table while ACT is idle so
        # LN2's real sqrt doesn't pay the 1.3us switch from exp.
        nc.scalar.activation(scr11, epst, AF.Sqrt)
        psS_ctx.close()

        if stop_after == 'attn':
            attn_ctx.close()
            xfull_ctx.close()
            return nc
        # ---------- phase 6: proj + residual (+proj bias) -> x2T ----------
        # LN2's channel sums are interleaved one ob behind the proj matmuls so
        # only the stats tail (~3us) remains after the last proj block.
        psC_ctx = ExitStack()
        psC = psC_ctx.enter_context(tc.tile_pool(name="psC", bufs=2, space="PSUM"))
        psB2_ctx = ExitStack()
        psB2 = psB2_ctx.enter_context(tc.tile_pool(name="psB2", bufs=4, space="PSUM"))
        ps_t1 = psC.tile([1, NQ], F32, tag="s2")
        ps_t2 = psC.tile([1, NQ], F32, tag="s2")
        x2T = persist.tile([P, CT, NQ], F32R, tag="x2T")

        def ln2_sums(ob):
            # all elementwise work on DVE: keeps ACT free of table-switching
            # functions between the post-attention dummy sqrt and LN2's sqrt
            sq2 = sqpool.tile([P, NQ], F32R, tag="sq", name=f"sq2_{ob}")
            nc.vector.tensor_tensor(sq2, x2T[:, ob, :].bitcast(F32),
                                    x2T[:, ob, :].bitcast(F32), ALU.mult)
            nc.tensor.matmul(ps_t1, ones_col, x2T[:, ob, :],
                             start=(ob == 0), stop=(ob == CT - 1))
            nc.tensor.matmul(ps_t2, ones_col, sq2,
                             start=(ob == 0), stop=(ob == CT - 1))

        for ob in range(CT):
            wp = wpool3.tile([P, CT, P], BF16, tag="wp")
            nc.sync.dma_start(wp, proj_wTt[ob])
            psp = psB2.tile([P, NQ], F32, tag="mm")
            for ct in range(CT):
                nc.tensor.matmul(psp, wp[:, ct], oT[:, ct, :],
                                 start=(ct == 0), stop=(ct == CT - 1))
            nc.vector.tensor_scalar(x2T[:, ob, :], psp, projb_sb[:, ob:ob + 1],
                                    None, ALU.add)
            nc.vector.tensor_tensor(x2T[:, ob, :], x2T[:, ob, :].bitcast(F32),
                                    xf[:, ob, 0:NQ].bitcast(F32), ALU.add)
            if ob > 0:
                ln2_sums(ob - 1)
        ln2_sums(CT - 1)

        attn_ctx.close()
        xfull_ctx.close()
        psB2_ctx.close()

        if stop_after == 'proj':
            psC_ctx.close()
            return nc
        # ---------- phase 7: LN2 stats + normalize ----------
        # prefetch the first two fc1 weight groups while the stats chain runs
        w1T = fc1_wT.rearrange("(ct p) m -> p ct m", p=P)
        w1_tiles = {}
        for hg in range(2):
            w1 = wpool.tile([P, CT, NQ], BF16, tag="w1", bufs=2, name=f"w1_{hg}")
            nc.sync.dma_start(w1, w1T[:, :, hg * 512:(hg + 1) * 512])
            w1_tiles[hg] = w1
        mu2 = rows2.tile([1, NQ], F32R, tag="r512ln")
        nc.vector.tensor_scalar_mul(mu2, ps_t1, 1.0 / C)
        mu2b = bcast.tile([P, NQ], F32, tag="b512")
        nc.gpsimd.partition_broadcast(mu2b[:, :], mu2.bitcast(F32)[:, :])
        e22 = rows2.tile([1, NQ], F32R, tag="r512ln")
        nc.vector.tensor_scalar_mul(e22, ps_t2, 1.0 / C)
        tmp2 = rows2.tile([1, NQ], F32R, tag="r512ln")
        nc.vector.tensor_tensor(tmp2, mu2.bitcast(F32), mu2.bitcast(F32), ALU.mult)
        nc.vector.tensor_tensor(e22, e22.bitcast(F32), tmp2.bitcast(F32), ALU.subtract)
        nc.scalar.activation(e22, e22.bitcast(F32), AF.Sqrt, bias=epst[:, :])
        with nc.allow_low_precision(reason="f32r rsig rounding ~1e-4 is fine"):
            nc.vector.reciprocal(e22, e22.bitcast(F32))
        rs2b = bcast.tile([P, NQ], F32, tag="b512")
        nc.gpsimd.partition_broadcast(rs2b[:, :], e22.bitcast(F32)[:, :])
        psC_ctx.close()
        h2T = hpool.tile([P, CT, NQ], BF16, tag="h0")
        h2tmp = hpool.tile([P, NQ], F32, tag="htmp0")
        for ob in range(CT):
            nc.vector.tensor_tensor(h2tmp, x2T[:, ob, :].bitcast(F32), mu2b,
                                    ALU.subtract)
            nc.vector.tensor_tensor(h2T[:, ob, :], h2tmp, rs2b, ALU.mult)

        if stop_after == 'ln2':
            return nc
        # ---------- phase 8a: fc1 + gelu -> m (bf16, resident in SBUF) ----------
        mlp_ctx = ExitStack()
        mpool = mlp_ctx.enter_context(tc.tile_pool(name="mpool", bufs=1))
        wpool2 = mlp_ctx.enter_context(tc.tile_pool(name="wpool2", bufs=2))
        psF_ctx = ExitStack()
        psF = psF_ctx.enter_context(tc.tile_pool(name="psF", bufs=8, space="PSUM"))
        m_sb = mpool.tile([P, HT, NQ], BF16, tag="m")
        w2_tiles = {}
        for hg in range(8):
            pss = [psF.tile([P, NQ], F32, tag="mm", name=f"ps_fc1_{hg}_{j}")
                   for j in range(4)]
            if hg in w1_tiles:
                w1 = w1_tiles[hg]
            else:
                w1 = wpool.tile([P, CT, NQ], BF16, tag="w1", bufs=2, name=f"w1_{hg}")
                nc.sync.dma_start(w1, w1T[:, :, hg * 512:(hg + 1) * 512])
            if hg == 4:
                # prefetch the first two fc2 weight blocks now: late enough
                # not to starve the w1 stream, early enough to hide fc2 start
                for ob in range(2):
                    w2 = wpool2.tile([P, HT, P], BF16, tag="w2", name=f"w2_{ob}")
                    nc.sync.dma_start(w2, fc2_wTt[ob])
                    w2_tiles[ob] = w2
            if hg < 7:
                for ct in range(CT):
                    for j in range(4):
                        nc.tensor.matmul(pss[j], w1[:, ct, j * P:(j + 1) * P],
                                         h2T[:, ct, :],
                                         start=(ct == 0), stop=(ct == CT - 1))
                for j in range(4):
                    hb = hg * 4 + j
                    nc.scalar.activation(m_sb[:, hb, :], pss[j], AF.Gelu,
                                         bias=fc1b_sb[:, hb:hb + 1])
            else:
                # last group: serialize per-j so the gelus drain while the
                # remaining chains still run (shrinks the psF release barrier)
                for j in range(4):
                    for ct in range(CT):
                        nc.tensor.matmul(pss[j], w1[:, ct, j * P:(j + 1) * P],
                                         h2T[:, ct, :],
                                         start=(ct == 0), stop=(ct == CT - 1))
                    hb = hg * 4 + j
                    nc.scalar.activation(m_sb[:, hb, :], pss[j], AF.Gelu,
                                         bias=fc1b_sb[:, hb:hb + 1])
        psF_ctx.close()

        if stop_after == 'fc1':
            mlp_ctx.close()
            return nc
        # ---------- phase 8b: fc2 + bias + residual -> out (ob-outer) ----------
        psD_ctx = ExitStack()
        psD = psD_ctx.enter_context(tc.tile_pool(name="psD", bufs=2, space="PSUM"))
        for ob in range(CT):
            if ob in w2_tiles:
                w2 = w2_tiles[ob]
            else:
                w2 = wpool2.tile([P, HT, P], BF16, tag="w2", name=f"w2_{ob}")
                nc.sync.dma_start(w2, fc2_wTt[ob])
            ot = outpool.tile([P, NQ], F32, tag="out")
            if ob < CT - 1:
                ps = psD.tile([P, NQ], F32, tag="fc2", name=f"ps_fc2_{ob}")
                for ht in range(HT):
                    nc.tensor.matmul(ps, w2[:, ht, :], m_sb[:, ht, :],
                                     start=(ht == 0), stop=(ht == HT - 1))
                nc.vector.tensor_scalar(ot, ps, fc2b_sb[:, ob:ob + 1], None, ALU.add)
                nc.vector.tensor_tensor(ot, ot, x2T[:, ob, :].bitcast(F32), ALU.add)
                nc.sync.dma_start(outT[ob * P:(ob + 1) * P, :], ot)
            else:
                # last block: two half-width psum chains so half A's epilogue
                # and store overlap half B's matmuls (shortens the tail)
                for hh in range(2):
                    hs = slice(hh * 256, (hh + 1) * 256)
                    ph = psD.tile([P, 256], F32, tag="fc2h", bufs=2,
                                  name=f"ps_fc2_l{hh}")
                    for ht in range(HT):
                        nc.tensor.matmul(ph, w2[:, ht, :], m_sb[:, ht, hs],
                                         start=(ht == 0), stop=(ht == HT - 1))
                    nc.vector.tensor_scalar(ot[:, hs], ph,
                                            fc2b_sb[:, ob:ob + 1], None, ALU.add)
                    nc.vector.tensor_tensor(ot[:, hs], ot[:, hs],
                                            x2T[:, ob, hs].bitcast(F32), ALU.add)
                    nc.sync.dma_start(outT[ob * P:(ob + 1) * P, hs], ot[:, hs])
        psD_ctx.close()
        mlp_ctx.close()

    return nc


# ---------------------------------------------------------------------------
# Host side: shard, run, gather
# ---------------------------------------------------------------------------
_RUNNER = None


class _Runner:
    """Minimal SPMD executor via bass2jax custom call (axon PJRT path)."""

    def __init__(self, nc, n_cores):
        import jax
        from jax.sharding import Mesh, PartitionSpec
        from jax.experimental.shard_map import shard_map
        from concourse.bass2jax import (_bass_exec_p, install_neuronx_cc_hook,
                                        partition_id_tensor)
        install_neuronx_cc_hook()
        self.jax = jax
        self.nc = nc
        self.n_cores = n_cores
        partition_name = nc.partition_id_tensor.name if nc.partition_id_tensor else None
        in_names, out_names, out_avals, zero_outs = [], [], [], []
        for alloc in nc.m.functions[0].allocations:
            if not isinstance(alloc, mybir.MemoryLocationSet):
                continue
            name = alloc.memorylocations[0].name
            if alloc.kind == "ExternalInput":
                if name != partition_name:
                    in_names.append(name)
            elif alloc.kind == "ExternalOutput":
                shape = tuple(alloc.tensor_shape)
                dtype = mybir.dt.np(alloc.dtype)
                out_names.append(name)
                out_avals.append(jax.core.ShapedArray(shape, dtype))
                zero_outs.append(np.zeros(shape, dtype))
        self.in_names, self.out_names = in_names, out_names
        self._out_avals, self._zero_outs = out_avals, zero_outs
        n_params = len(in_names)
        all_in = in_names + out_names + ([partition_name] if partition_name else [])

        def _body(*args):
            operands = list(args)
            if partition_name is not None:
                operands.append(partition_id_tensor())
            return tuple(_bass_exec_p.bind(
                *operands, out_avals=tuple(out_avals), in_names=tuple(all_in),
                out_names=tuple(out_names), lowering_input_output_aliases=(),
                sim_require_finite=True, sim_require_nnan=True, nc=nc))

        devices = jax.devices()[:n_cores]
        mesh = Mesh(np.asarray(devices), ("core",))
        nspec = n_params + len(out_names)
        self._fn = jax.jit(
            shard_map(_body, mesh=mesh, in_specs=(PartitionSpec("core"),) * nspec,
                      out_specs=(PartitionSpec("core"),) * len(out_names),
                      check_rep=False),
            keep_unused=True)

    def run(self, in_maps):
        n = self.n_cores
        per_core = [[np.ascontiguousarray(m[k]) for k in self.in_names] for m in in_maps]
        args = [np.concatenate([per_core[c][i] for c in range(n)], axis=0)
                for i in range(len(self.in_names))]
        args += [np.zeros((n * z.shape[0], *z.shape[1:]), z.dtype) for z in self._zero_outs]
        outs = self._fn(*args)
        self.jax.block_until_ready(outs)
        return [
            {name: np.asarray(outs[i]).reshape(n, *self._out_avals[i].shape)[c]
             for i, name in enumerate(self.out_names)}
            for c in range(n)
        ]


def _get_runner():
    global _RUNNER
    if _RUNNER is None:
        _RUNNER = _Runner(build_nc(), 8)
    return _RUNNER


def kernel(x, mask, ln1_g, ln1_b, qkv_w, qkv_b, proj_w, proj_b,
           ln2_g, ln2_b, fc1_w, fc1_b, fc2_w, fc2_b):
    x = np.asarray(x, np.float32)
    mask = np.asarray(mask, bool)
    ln1_g = np.asarray(ln1_g, np.float32); ln1_b = np.asarray(ln1_b, np.float32)
    qkv_w = np.asarray(qkv_w, np.float32); qkv_b = np.asarray(qkv_b, np.float32)
    proj_w = np.asarray(proj_w, np.float32); proj_b = np.asarray(proj_b, np.float32)
    ln2_g = np.asarray(ln2_g, np.float32); ln2_b = np.asarray(ln2_b, np.float32)
    fc1_w = np.asarray(fc1_w, np.float32); fc1_b = np.asarray(fc1_b, np.float32)
    fc2_w = np.asarray(fc2_w, np.float32); fc2_b = np.asarray(fc2_b, np.float32)

    # fold LN1 gain/bias + softmax scale into qkv weights/bias
    Wq = qkv_w * ln1_g[None, :]
    qb = qkv_w @ ln1_b + qkv_b
    Wq[:C] *= SCALE
    qb = qb.copy(); qb[:C] *= SCALE
    qkv_wT = np.ascontiguousarray(Wq.T).astype(ml_dtypes.bfloat16)
    # fold LN2 gain/bias into fc1
    W1 = fc1_w * ln2_g[None, :]
    fb1 = fc1_w @ ln2_b + fc1_b
    fc1_wT = np.ascontiguousarray(W1.T).astype(ml_dtypes.bfloat16)
    # proj/fc2 weights pre-tiled as [out_blk, partition, in_blk, out_col] so
    # each per-block DMA is one fully-contiguous 2KB+/partition transfer.
    proj_wTt = np.ascontiguousarray(
        proj_w.T.reshape(CT, P, CT, P).transpose(2, 1, 0, 3)
    ).astype(ml_dtypes.bfloat16)
    fc2_wTt = np.ascontiguousarray(
        fc2_w.T.reshape(HT, P, CT, P).transpose(2, 1, 0, 3)
    ).astype(ml_dtypes.bfloat16)
    qkvb_v = np.ascontiguousarray(qb[2 * C:].reshape(1, C)).astype(ml_dtypes.bfloat16)
    maskmul = (~mask).astype(ml_dtypes.bfloat16)

    in_maps = []
    for core in range(8):
        b, s = core // 2, core % 2
        # roll x so this core's query half is always tokens [0:512]
        xb = x[b]                         # [N, C]
        xroll = np.roll(xb, -s * NQ, axis=0)
        xT_c = np.ascontiguousarray(xroll.T)               # [C, NK]
        # mask rows q = this core's queries; key order must match rolled order
        mrow = maskmul[b, s * NQ:(s + 1) * NQ]             # [NQ, N] keys orig order
        mroll = np.roll(mrow, -s * NQ, axis=1)             # keys in rolled order
        maskT_c = np.ascontiguousarray(mroll.T)            # [NK, NQ]
        in_maps.append({
            "ones_d": np.ones(P, np.float32),
            "xT": xT_c.astype(np.float32),
            "maskT": maskT_c,
            "qkv_wT": qkv_wT, "qkvb_qk": qb[:2 * C].copy(), "qkvb_v": qkvb_v,
            "proj_wTt": proj_wTt, "projb": proj_b.copy(),
            "fc1_wT": fc1_wT, "fc1b": fb1.copy(),
            "fc2_wTt": fc2_wTt, "fc2b": fc2_b.copy(),
        })

    results = _get_runner().run(in_maps)
    out = np.empty((B, N, C), np.float32)
    for core in range(8):
        b, s = core // 2, core % 2
        out[b, s * NQ:(s + 1) * NQ, :] = results[core]["outT"].T
    return out


# revision 26
# speedup vs baseline: 1.3066x; 1.1466x over previous
"""Trainium2 Bass kernel for a pre-norm transformer block (B=4, N=1024, C=1024,
16 heads, MLP hidden 4096), SPMD across 8 NeuronCores.

Sharding: core = (b, s) with b = batch element (4), s = query-half (2).
Each core computes the block for 512 query tokens of one batch element:
  - LN1 + K/V projections over the full 1024-token sequence of its batch
    element (duplicated within the batch pair - avoids all collectives),
  - Q projection + attention + proj + residual + LN2 + MLP for its 512 rows.

Everything on-device is channel-major (channels on partitions, tokens on the
free dim) so no on-device transposes are needed; the host passes x.T and
pre-transposed weights. LayerNorm gains/biases are folded into the following
matmul weights/biases on the host; the softmax scale is folded into the Q
weights. Softmax uses no max-subtraction (logits are O(1) by construction),
which makes the softmax1 "+1" denominator term exact and free; the
denominator itself comes from a ones-column appended to V inside the P@V
matmul.

Schedule notes (v2): x is DMA'd once into a resident f32 tile (freed before
the MLP so its SBUF is reused for the fc1 activations, which never touch
DRAM); the mask load rides the Pool/SWDGE queue so it doesn't delay the x
stream; QKV runs in 8-PSUM-bank rounds so K weights load once; proj/fc2
weights are pre-tiled on the host so every DMA descriptor is >=2KB; fc2 runs
output-block-outer so the bias+residual+store epilogue of block i overlaps
the matmuls of block i+1.

Precision: main matmuls in float32r (full PE rate, ~1e-4 matmul rel err);
attention S/P/O and the MLP hidden activations in bf16; everything else fp32.
"""

from contextlib import ExitStack

import numpy as np
import ml_dtypes

import concourse.bass as bass
import concourse.mybir as mybir
import concourse.tile as tile
from concourse import bacc

P = 128
B, N, C = 4, 1024, 1024
H, D = 16, 64
HD = 4 * C
NQ = N // 2          # query tokens per core
NK = N               # key/value tokens per core
CT = C // P          # 8 channel tiles
HT = HD // P         # 32 hidden tiles
KB = NK // P         # 8 key-token tiles
SCALE = D ** (-0.5)
EPS = 1e-5

F32 = mybir.dt.float32
F32R = mybir.dt.float32r
BF16 = mybir.dt.bfloat16
F8 = mybir.dt.float8e4
DR = mybir.MatmulPerfMode.DoubleRow
AF = mybir.ActivationFunctionType
ALU = mybir.AluOpType


def build_nc(stop_after=None):
    nc = _build_body(stop_after)
    nc.compile()
    return nc


def _build_body(stop_after=None):
    nc = bacc.Bacc("TRN2", target_bir_lowering=False, debug=False, num_devices=8)

    # ---- DRAM I/O (per core) ----
    xT = nc.dram_tensor("xT", [C, NK], F32R, kind="ExternalInput")
    ones_d = nc.dram_tensor("ones_d", [P], F32R, kind="ExternalInput")
    maskT = nc.dram_tensor("maskT", [NK, NQ], BF16, kind="ExternalInput")  # 1-mask, transposed
    qkv_wT = nc.dram_tensor("qkv_wT", [C, 3 * C], F8, kind="ExternalInput")
    qkvb_qk = nc.dram_tensor("qkvb_qk", [2 * C], F32, kind="ExternalInput")
    qkvb_v = nc.dram_tensor("qkvb_v", [1, C], BF16, kind="ExternalInput")
    proj_wTt = nc.dram_tensor("proj_wTt", [CT, P, CT, P], F8, kind="ExternalInput")
    scales = nc.dram_tensor("scales", [P, 4], F32, kind="ExternalInput")
    projb = nc.dram_tensor("projb", [C], F32, kind="ExternalInput")
    fc1_wT = nc.dram_tensor("fc1_wT", [C, HD], BF16, kind="ExternalInput")
    fc1b = nc.dram_tensor("fc1b", [HD], F32, kind="ExternalInput")
    fc2_wTt = nc.dram_tensor("fc2_wTt", [CT, P, HT, P], BF16, kind="ExternalInput")
    fc2b = nc.dram_tensor("fc2b", [C], F32, kind="ExternalInput")
    outT = nc.dram_tensor("outT", [C, NQ], F32, kind="ExternalOutput")

    with tile.TileContext(nc) as tc, ExitStack() as ctx:
        persist = ctx.enter_context(tc.tile_pool(name="persist", bufs=1))
        hpool = ctx.enter_context(tc.tile_pool(name="hpool", bufs=1))
        sqpool = ctx.enter_context(tc.tile_pool(name="sqpool", bufs=2))
        wpool = ctx.enter_context(tc.tile_pool(name="wpool", bufs=4))
        vwpool = ctx.enter_context(tc.tile_pool(name="vwpool", bufs=8))
        wpool3 = ctx.enter_context(tc.tile_pool(name="wpool3", bufs=2))
        ptpool = ctx.enter_context(tc.tile_pool(name="ptpool", bufs=3))
        rows2 = ctx.enter_context(tc.tile_pool(name="rows2", bufs=3))
        bcast = ctx.enter_context(tc.tile_pool(name="bcast", bufs=2))
        rbpool = ctx.enter_context(tc.tile_pool(name="rbpool", bufs=2))
        outpool = ctx.enter_context(tc.tile_pool(name="outpool", bufs=2))

        # scoped: x (f32, 32KB/partition) lives until the proj residual; the
        # attention tensors live until proj. Both release before the MLP pools
        # allocate, so SBUF peaks stay under the per-partition budget.
        xfull_ctx = ExitStack()
        xpool = xfull_ctx.enter_context(tc.tile_pool(name="xpool", bufs=1))
        attn_ctx = ExitStack()
        apool = attn_ctx.enter_context(tc.tile_pool(name="apool", bufs=1))

        # ---------- startup DMAs: ones (for LN sums), then x, then consts ----
        ones_col = persist.tile([P, 1], F32R, tag="ones_col")
        nc.sync.dma_start(ones_col, ones_d.rearrange("(p o) -> p o", o=1))

        xf = xpool.tile([P, CT, NK], F32R, tag="xf")
        xTr = xT.rearrange("(ct p) n -> p ct n", p=P)
        # first two cts as single-ct chunks so the very first sum/square can
        # start ~0.7us earlier; the rest as 2-ct chunks
        nc.sync.dma_start(xf[:, 0:1, 0:NQ], xTr[:, 0:1, 0:NQ])
        nc.sync.dma_start(xf[:, 1:2, 0:NQ], xTr[:, 1:2, 0:NQ])
        for t in range(1, CT // 2):
            nc.sync.dma_start(xf[:, 2 * t:2 * t + 2, 0:NQ], xTr[:, 2 * t:2 * t + 2, 0:NQ])
        for t in range(CT // 2):
            nc.sync.dma_start(xf[:, 2 * t:2 * t + 2, NQ:NK], xTr[:, 2 * t:2 * t + 2, NQ:NK])

        epst = persist.tile([1, 1], F32, tag="eps")
        nc.vector.memset(epst, EPS)
        # dummy sqrt: pulls the sqrt act-table load off LN1's critical path
        # (the set also contains square/identity/copy, so nothing else loads
        # a table until attention's exp).
        scr11 = persist.tile([1, 1], F32, tag="scr")
        nc.scalar.activation(scr11, epst, AF.Sqrt)
        qkb_sb = persist.tile([P, 16], F32, tag="qkb")
        nc.sync.dma_start(qkb_sb, qkvb_qk.rearrange("(o p) -> p o", p=P))
        fc1b_sb = persist.tile([P, HT], F32, tag="fc1b")
        nc.sync.dma_start(fc1b_sb, fc1b.rearrange("(o p) -> p o", p=P))
        fc2b_sb = persist.tile([P, CT], F32, tag="fc2b")
        nc.sync.dma_start(fc2b_sb, fc2b.rearrange("(o p) -> p o", p=P))
        projb_sb = persist.tile([P, CT], F32, tag="projb")
        nc.sync.dma_start(projb_sb, projb.rearrange("(o p) -> p o", p=P))
        sc_sb = persist.tile([P, 4], F32, tag="scales")
        nc.sync.dma_start(sc_sb, scales.ap())
        vb_row = persist.tile([1, C], BF16, tag="vb")
        nc.sync.dma_start(vb_row, qkvb_v.ap())
        ones_k1 = persist.tile([1, P], BF16, tag="ones_k1")
        nc.gpsimd.dma_start(ones_k1, ones_d.rearrange("(o p) -> o p", o=1))
        # consts for folding softmax1's "+1" denominator term into the P@V
        # psum as a rank-1 matmul (e65 x ones_q adds 1.0 to the denom row)
        e65 = persist.tile([1, 65], BF16, tag="e65")
        nc.vector.memset(e65[:, 0:64], 0.0)
        nc.vector.memset(e65[:, 64:65], 1.0)
        ones_q = persist.tile([1, NQ], BF16, tag="ones_q")
        nc.vector.memset(ones_q, 1.0)

        # mask (bf16, [k, q] as [128, kb, q]) on the Pool/SWDGE queue so it
        # never delays the x / weight streams on the HWDGE path.
        mask_sb = apool.tile([P, KB, NQ], BF16, tag="mask")
        nc.gpsimd.dma_start(mask_sb, maskT.rearrange("(kb p) q -> p kb q", p=P))

        # ---------- phase 1+2: LN1 (sums -> stats -> normalize per half) ----
        hTh = []
        psA_ctx = ExitStack()
        psA = psA_ctx.enter_context(tc.tile_pool(name="psA", bufs=2, space="PSUM"))
        stats = []
        for nh in range(2):
            sl = slice(nh * NQ, (nh + 1) * NQ)
            ps_s1 = psA.tile([1, NQ], F32, tag="s", name=f"ps_s1_{nh}")
            ps_s2 = psA.tile([1, NQ], F32, tag="s2", name=f"ps_s2_{nh}")
            for ct in range(CT):
                sq = sqpool.tile([P, NQ], F32R, tag="sq", name=f"sq_{nh}_{ct}")
                xc = xf[:, ct, sl]
                if ct % 2 == 0:
                    nc.scalar.activation(sq, xc.bitcast(F32), AF.Square)
                else:
                    nc.vector.tensor_tensor(sq, xc.bitcast(F32), xc.bitcast(F32),
                                            ALU.mult)
                nc.tensor.matmul(ps_s1, ones_col, xc,
                                 start=(ct == 0), stop=(ct == CT - 1))
                nc.tensor.matmul(ps_s2, ones_col, sq,
                                 start=(ct == 0), stop=(ct == CT - 1))
            stats.append((ps_s1, ps_s2))
        for nh in range(2):
            sl = slice(nh * NQ, (nh + 1) * NQ)
            ps_s1, ps_s2 = stats[nh]
            mu_row = rows2.tile([1, NQ], F32, tag="r512ln", name=f"mu_{nh}")
            nc.vector.tensor_scalar_mul(mu_row, ps_s1, 1.0 / C)
            e2_row = rows2.tile([1, NQ], F32, tag="r512ln", name=f"e2_{nh}")
            nc.vector.tensor_scalar_mul(e2_row, ps_s2, 1.0 / C)
            tmp_row = rows2.tile([1, NQ], F32, tag="r512ln", name=f"tmp_{nh}")
            nc.vector.tensor_tensor(tmp_row, mu_row, mu_row, ALU.mult)
            nc.vector.tensor_tensor(e2_row, e2_row, tmp_row, ALU.subtract)
            nc.scalar.activation(e2_row, e2_row, AF.Sqrt, bias=epst[:, :])
            nc.vector.reciprocal(e2_row, e2_row)
            mu_bc = bcast.tile([P, NQ], F32, tag="b512", name=f"mu_b_{nh}")
            nc.gpsimd.partition_broadcast(mu_bc[:, :], mu_row[:, :])
            rs_bc = bcast.tile([P, NQ], F32, tag="b512", name=f"rs_b_{nh}")
            nc.gpsimd.partition_broadcast(rs_bc[:, :], e2_row[:, :])
            hT = hpool.tile([P, CT, NQ], F8, tag=f"h{nh}")
            tmpf = hpool.tile([P, NQ], F32, tag=f"htmp{nh}")
            for ct in range(CT):
                nc.vector.tensor_tensor(tmpf, xf[:, ct, sl].bitcast(F32), mu_bc,
                                        ALU.subtract)
                nc.vector.tensor_tensor(hT[:, ct, :], tmpf, rs_bc, ALU.mult)
            hTh.append(hT)
        psA_ctx.close()

        if stop_after == 'ln1':
            attn_ctx.close()
            xfull_ctx.close()
            return nc
        # ---------- phase 3: q^T, k^T in 8-PSUM-bank rounds ----------
        # Host passes xT ROLLED per core: the query half is always tokens
        # [0:512] (= hTh[0]); K/V cover both halves with mask columns matched.
        qT = apool.tile([P, CT, NQ], BF16, tag="qT")
        kT = apool.tile([P, CT, NK], BF16, tag="kT")
        wT = qkv_wT.rearrange("(ct p) m -> p ct m", p=P)
        # psV allocated BEFORE psB so the V psums own banks that the Q/K bias
        # drains never touch - V matmuls start the moment K matmuls end.
        psV_ctx = ExitStack()
        psV = psV_ctx.enter_context(tc.tile_pool(name="psV", bufs=4, space="PSUM"))
        psB_ctx = ExitStack()
        psB = psB_ctx.enter_context(tc.tile_pool(name="psB", bufs=4, space="PSUM"))
        # Q: two 4-psum rounds sharing one set of weight tiles (loaded once)
        wq_tiles = []
        for t in range(CT // 2):
            wt = wpool.tile([P, 2, 2 * NQ], F8, tag="w", name=f"wq_{t}")
            nc.sync.dma_start(wt, wT[:, 2 * t:2 * t + 2, 0:C])
            wq_tiles.append(wt)
        for qr in range(2):
            pss = [psB.tile([P, NQ], F32, tag="mm", name=f"ps_q{qr}_{j}")
                   for j in range(4)]
            for t in range(CT // 2):
                for j in range(4):
                    jj = qr * 4 + j
                    nc.tensor.matmul(pss[j], wq_tiles[t][:, :, jj * P:(jj + 1) * P],
                                     hTh[0][:, 2 * t:2 * t + 2, :], perf_mode=DR,
                                     start=(t == 0), stop=(t == CT // 2 - 1))
            for j in range(4):
                jj = qr * 4 + j
                nc.scalar.activation(qT[:, jj, :], pss[j], AF.Identity,
                                     bias=qkb_sb[:, jj:jj + 1], scale=sc_sb[:, 0:1])
        # K: two weight groups x two 4-psum rounds; biases split ACT/DVE
        for kr in range(2):
            wk_tiles = []
            for t in range(CT // 2):
                wt = wpool.tile([P, 2, NQ], F8, tag="w", name=f"wk{kr}_{t}")
                nc.sync.dma_start(wt, wT[:, 2 * t:2 * t + 2,
                                         C + kr * 512:C + (kr + 1) * 512])
                wk_tiles.append(wt)
            for half in range(2):
                pss = [psB.tile([P, NQ], F32, tag="mm", name=f"ps_k{kr}_{half}_{x}")
                       for x in range(4)]
                for t in range(CT // 2):
                    for j2 in range(2):
                        for nh in range(2):
                            j = half * 2 + j2
                            nc.tensor.matmul(
                                pss[2 * j2 + nh],
                                wk_tiles[t][:, :, j * P:(j + 1) * P],
                                hTh[nh][:, 2 * t:2 * t + 2, :], perf_mode=DR,
                                start=(t == 0), stop=(t == CT // 2 - 1))
                for j2 in range(2):
                    j = half * 2 + j2
                    mb = 8 + kr * 4 + j
                    nc.scalar.activation(
                        kT[:, mb - 8, 0:NQ], pss[2 * j2],
                        AF.Identity, bias=qkb_sb[:, mb:mb + 1], scale=sc_sb[:, 0:1])
                    nc.vector.tensor_scalar(
                        kT[:, mb - 8, NQ:NK], pss[2 * j2 + 1],
                        sc_sb[:, 0:1], qkb_sb[:, mb:mb + 1], ALU.mult, ALU.add)
        psB_ctx.close()

        if stop_after == 'qk':
            psV_ctx.close()
            attn_ctx.close()
            xfull_ctx.close()
            return nc
        # ---------- phase 4: v token-major + ones column ----------
        # V_aug: [128 tok, kb, head, 65] (65th col = 1.0 for denominator)
        V_aug = apool.tile([P, KB, H, 65], BF16, tag="vaug")
        nc.vector.memset(V_aug[:, :, :, 64:65], 1.0)
        for nhp in range(2):
            wvs = []
            for t in range(CT // 2):
                wv = vwpool.tile([P, 2, NQ], F8, tag="vw", name=f"vw_{nhp}_{t}")
                nc.sync.dma_start(wv, wT[:, 2 * t:2 * t + 2,
                                         2048 + nhp * 512: 2048 + (nhp + 1) * 512])
                wvs.append(wv)
            for nh2 in range(2):
                nh = 2 * nhp + nh2
                vsl = slice(nh2 * 256, (nh2 + 1) * 256)
                for tb in range(KB):
                    hsrc = hTh[tb // 4][:, :, (tb % 4) * P:(tb % 4 + 1) * P]
                    psv = psV.tile([P, 256], F32, tag="mmv", name=f"psv_{nh}_{tb}")
                    for t in range(CT // 2):
                        nc.tensor.matmul(psv, hsrc[:, 2 * t:2 * t + 2, :],
                                         wvs[t][:, :, vsl], perf_mode=DR,
                                         start=(t == 0), stop=False)
                    nc.tensor.matmul(psv, ones_k1,
                                     vb_row[:, nh * 256:(nh + 1) * 256],
                                     start=False, stop=True)
                    # drain copies alternate ACT/DVE; scale 4/s_qkv makes
                    # V_aug = 4*v (the 4 rides through P@V into oT's fp8)
                    if tb % 2 == 0:
                        nc.scalar.activation(
                            V_aug[:, tb, nh * 4:(nh + 1) * 4, 0:64],
                            psv.rearrange("p (h d) -> p h d", d=64),
                            AF.Copy, scale=sc_sb[:, 1:2])
                    else:
                        nc.vector.tensor_scalar(
                            V_aug[:, tb, nh * 4:(nh + 1) * 4, 0:64],
                            psv.rearrange("p (h d) -> p h d", d=64),
                            sc_sb[:, 1:2], None, ALU.mult)
        psV_ctx.close()

        if stop_after == 'v':
            attn_ctx.close()
            xfull_ctx.close()
            return nc
        # ---------- phase 5: attention per head ----------
        # S psums in [128, 2, 512] groups -> 1024-wide exp and mask ops.
        psS_ctx = ExitStack()
        psS = psS_ctx.enter_context(tc.tile_pool(name="psS", bufs=3, space="PSUM"))
        psO = psS_ctx.enter_context(tc.tile_pool(name="psO", bufs=2, space="PSUM"))
        oT = apool.tile([P, CT, NQ], F8, tag="oT")
        for h in range(H):
            j, base = h // 2, (h % 2) * 64
            po = psO.tile([65, NQ], F32, tag="o")
            for g in range(4):
                psg = psS.tile([P, 2, NQ], F32, tag="sg", name=f"psg_{h}_{g}")
                for i in range(2):
                    kb = 2 * g + i
                    nc.tensor.matmul(psg[:, i, :],
                                     kT[base:base + 64, j, kb * P:(kb + 1) * P],
                                     qT[base:base + 64, j, :], start=True, stop=True)
                PT = ptpool.tile([P, 2, NQ], BF16, tag="pt", name=f"pt_{h}_{g}")
                nc.scalar.activation(PT, psg, AF.Exp)
                nc.vector.tensor_tensor(PT, PT, mask_sb[:, 2 * g:2 * g + 2, :], ALU.mult)
                for i in range(2):
                    kb = 2 * g + i
                    nc.tensor.matmul(po, V_aug[:, kb, h, :], PT[:, i, :],
                                     start=(kb == 0), stop=False)
            nc.tensor.matmul(po, e65, ones_q, start=False, stop=True)
            drow = rows2.tile([1, NQ], F32, tag="r512", bufs=2)
            nc.vector.reciprocal(drow, po[64:65, :])
            rb = rbpool.tile([64, NQ], F32, tag="rb")
            nc.gpsimd.partition_broadcast(rb[:, :], drow[:, :])
            nc.vector.tensor_tensor(oT[base:base + 64, j, :], po[0:64, :], rb, ALU.mult)
        # dummy sqrt emitted inside the attention scope (before the pool
        # release barrier): reloads the sqrt act-table while ACT is idle so
        # LN2's real sqrt doesn't pay the 1.3us switch from exp.
        nc.scalar.activation(scr11, epst, AF.Sqrt)
        psS_ctx.close()

        if stop_after == 'attn':
            attn_ctx.close()
            xfull_ctx.close()
            return nc
        # ---------- phase 6: proj + residual (+proj bias) -> x2T ----------
        # LN2's channel sums are interleaved one ob behind the proj matmuls so
        # only the stats tail (~3us) remains after the last proj block.
        psC_ctx = ExitStack()
        psC = psC_ctx.enter_context(tc.tile_pool(name="psC", bufs=2, space="PSUM"))
        psB2_ctx = ExitStack()
        psB2 = psB2_ctx.enter_context(tc.tile_pool(name="psB2", bufs=4, space="PSUM"))
        ps_t1 = psC.tile([1, NQ], F32, tag="s2")
        ps_t2 = psC.tile([1, NQ], F32, tag="s2")
        x2T = persist.tile([P, CT, NQ], F32R, tag="x2T")

        def ln2_sums(ob):
            # all elementwise work on DVE: keeps ACT free of table-switching
            # functions between the post-attention dummy sqrt and LN2's sqrt
            sq2 = sqpool.tile([P, NQ], F32R, tag="sq", name=f"sq2_{ob}")
            nc.vector.tensor_tensor(sq2, x2T[:, ob, :].bitcast(F32),
                                    x2T[:, ob, :].bitcast(F32), ALU.mult)
            nc.tensor.matmul(ps_t1, ones_col, x2T[:, ob, :],
                             start=(ob == 0), stop=(ob == CT - 1))
            nc.tensor.matmul(ps_t2, ones_col, sq2,
                             start=(ob == 0), stop=(ob == CT - 1))

        for ob in range(CT):
            wp = wpool3.tile([P, CT, P], F8, tag="wp")
            nc.sync.dma_start(wp, proj_wTt[ob])
            psp = psB2.tile([P, NQ], F32, tag="mm")
            for t in range(CT // 2):
                nc.tensor.matmul(psp, wp[:, 2 * t:2 * t + 2, :],
                                 oT[:, 2 * t:2 * t + 2, :], perf_mode=DR,
                                 start=(t == 0), stop=(t == CT // 2 - 1))
            nc.vector.tensor_scalar(x2T[:, ob, :], psp, sc_sb[:, 2:3],
                                    projb_sb[:, ob:ob + 1], ALU.mult, ALU.add)
            nc.vector.tensor_tensor(x2T[:, ob, :], x2T[:, ob, :].bitcast(F32),
                                    xf[:, ob, 0:NQ].bitcast(F32), ALU.add)
            if ob > 0:
                ln2_sums(ob - 1)
        ln2_sums(CT - 1)

        attn_ctx.close()
        xfull_ctx.close()
        psB2_ctx.close()

        if stop_after == 'proj':
            psC_ctx.close()
            return nc
        # ---------- phase 7: LN2 stats + normalize ----------
        # prefetch the first two fc1 weight groups while the stats chain runs
        w1T = fc1_wT.rearrange("(ct p) m -> p ct m", p=P)
        w1_tiles = {}
        for hg in range(2):
            w1 = wpool.tile([P, CT, NQ], BF16, tag="w1", bufs=2, name=f"w1_{hg}")
            nc.sync.dma_start(w1, w1T[:, :, hg * 512:(hg + 1) * 512])
            w1_tiles[hg] = w1
        mu2 = rows2.tile([1, NQ], F32R, tag="r512ln")
        nc.vector.tensor_scalar_mul(mu2, ps_t1, 1.0 / C)
        mu2b = bcast.tile([P, NQ], F32, tag="b512")
        nc.gpsimd.partition_broadcast(mu2b[:, :], mu2.bitcast(F32)[:, :])
        e22 = rows2.tile([1, NQ], F32R, tag="r512ln")
        nc.vector.tensor_scalar_mul(e22, ps_t2, 1.0 / C)
        tmp2 = rows2.tile([1, NQ], F32R, tag="r512ln")
        nc.vector.tensor_tensor(tmp2, mu2.bitcast(F32), mu2.bitcast(F32), ALU.mult)
        nc.vector.tensor_tensor(e22, e22.bitcast(F32), tmp2.bitcast(F32), ALU.subtract)
        nc.scalar.activation(e22, e22.bitcast(F32), AF.Sqrt, bias=epst[:, :])
        with nc.allow_low_precision(reason="f32r rsig rounding ~1e-4 is fine"):
            nc.vector.reciprocal(e22, e22.bitcast(F32))
        rs2b = bcast.tile([P, NQ], F32, tag="b512")
        nc.gpsimd.partition_broadcast(rs2b[:, :], e22.bitcast(F32)[:, :])
        psC_ctx.close()
        h2T = hpool.tile([P, CT, NQ], BF16, tag="h0")
        h2tmp = hpool.tile([P, NQ], F32, tag="htmp0")
        for ob in range(CT):
            nc.vector.tensor_tensor(h2tmp, x2T[:, ob, :].bitcast(F32), mu2b,
                                    ALU.subtract)
            nc.vector.tensor_tensor(h2T[:, ob, :], h2tmp, rs2b, ALU.mult)

        if stop_after == 'ln2':
            return nc
        # ---------- phase 8a: fc1 + gelu -> m (bf16, resident in SBUF) ----------
        mlp_ctx = ExitStack()
        mpool = mlp_ctx.enter_context(tc.tile_pool(name="mpool", bufs=1))
        wpool2 = mlp_ctx.enter_context(tc.tile_pool(name="wpool2", bufs=2))
        psF_ctx = ExitStack()
        psF = psF_ctx.enter_context(tc.tile_pool(name="psF", bufs=8, space="PSUM"))
        m_sb = mpool.tile([P, HT, NQ], BF16, tag="m")
        w2_tiles = {}
        for hg in range(8):
            pss = [psF.tile([P, NQ], F32, tag="mm", name=f"ps_fc1_{hg}_{j}")
                   for j in range(4)]
            if hg in w1_tiles:
                w1 = w1_tiles[hg]
            else:
                w1 = wpool.tile([P, CT, NQ], BF16, tag="w1", bufs=2, name=f"w1_{hg}")
                nc.sync.dma_start(w1, w1T[:, :, hg * 512:(hg + 1) * 512])
            if hg == 4:
                # prefetch the first two fc2 weight blocks now: late enough
                # not to starve the w1 stream, early enough to hide fc2 start
                for ob in range(2):
                    w2 = wpool2.tile([P, HT, P], BF16, tag="w2", name=f"w2_{ob}")
                    nc.sync.dma_start(w2, fc2_wTt[ob])
                    w2_tiles[ob] = w2
            if hg < 7:
                for ct in range(CT):
                    for j in range(4):
                        nc.tensor.matmul(pss[j], w1[:, ct, j * P:(j + 1) * P],
                                         h2T[:, ct, :],
                                         start=(ct == 0), stop=(ct == CT - 1))
                for j in range(4):
                    hb = hg * 4 + j
                    nc.scalar.activation(m_sb[:, hb, :], pss[j], AF.Gelu,
                                         bias=fc1b_sb[:, hb:hb + 1])
            else:
                # last group: serialize per-j so the gelus drain while the
                # remaining chains still run (shrinks the psF release barrier)
                for j in range(4):
                    for ct in range(CT):
                        nc.tensor.matmul(pss[j], w1[:, ct, j * P:(j + 1) * P],
                                         h2T[:, ct, :],
                                         start=(ct == 0), stop=(ct == CT - 1))
                    hb = hg * 4 + j
                    nc.scalar.activation(m_sb[:, hb, :], pss[j], AF.Gelu,
                                         bias=fc1b_sb[:, hb:hb + 1])
        psF_ctx.close()

        if stop_after == 'fc1':
            mlp_ctx.close()
            return nc
        # ---------- phase 8b: fc2 + bias + residual -> out (ob-outer) ----------
        psD_ctx = ExitStack()
        psD = psD_ctx.enter_context(tc.tile_pool(name="psD", bufs=2, space="PSUM"))
        for ob in range(CT):
            if ob in w2_tiles:
                w2 = w2_tiles[ob]
            else:
                w2 = wpool2.tile([P, HT, P], BF16, tag="w2", name=f"w2_{ob}")
                nc.sync.dma_start(w2, fc2_wTt[ob])
            ot = outpool.tile([P, NQ], F32, tag="out")
            if ob < CT - 1:
                ps = psD.tile([P, NQ], F32, tag="fc2", name=f"ps_fc2_{ob}")
                for ht in range(HT):
                    nc.tensor.matmul(ps, w2[:, ht, :], m_sb[:, ht, :],
                                     start=(ht == 0), stop=(ht == HT - 1))
                nc.vector.tensor_scalar(ot, ps, fc2b_sb[:, ob:ob + 1], None, ALU.add)
                nc.vector.tensor_tensor(ot, ot, x2T[:, ob, :].bitcast(F32), ALU.add)
                nc.sync.dma_start(outT[ob * P:(ob + 1) * P, :], ot)
            else:
                # last block: two half-width psum chains so half A's epilogue
                # and store overlap half B's matmuls (shortens the tail)
                for hh in range(2):
                    hs = slice(hh * 256, (hh + 1) * 256)
                    ph = psD.tile([P, 256], F32, tag="fc2h", bufs=2,
                                  name=f"ps_fc2_l{hh}")
                    for ht in range(HT):
                        nc.tensor.matmul(ph, w2[:, ht, :], m_sb[:, ht, hs],
                                         start=(ht == 0), stop=(ht == HT - 1))
                    nc.vector.tensor_scalar(ot[:, hs], ph,
                                            fc2b_sb[:, ob:ob + 1], None, ALU.add)
                    nc.vector.tensor_tensor(ot[:, hs], ot[:, hs],
                                            x2T[:, ob, hs].bitcast(F32), ALU.add)
                    nc.sync.dma_start(outT[ob * P:(ob + 1) * P, hs], ot[:, hs])
        psD_ctx.close()
        mlp_ctx.close()

    return nc


# ---------------------------------------------------------------------------
# Host side: shard, run, gather
# ---------------------------------------------------------------------------
_RUNNER = None


class _Runner:
    """Minimal SPMD executor via bass2jax custom call (axon PJRT path)."""

    def __init__(self, nc, n_cores):
        import jax
        from jax.sharding import Mesh, PartitionSpec
        from jax.experimental.shard_map import shard_map
        from concourse.bass2jax import (_bass_exec_p, install_neuronx_cc_hook,
                                        partition_id_tensor)
        install_neuronx_cc_hook()
        self.jax = jax
        self.nc = nc
        self.n_cores = n_cores
        partition_name = nc.partition_id_tensor.name if nc.partition_id_tensor else None
        in_names, out_names, out_avals, zero_outs = [], [], [], []
        for alloc in nc.m.functions[0].allocations:
            if not isinstance(alloc, mybir.MemoryLocationSet):
                continue
            name = alloc.memorylocations[0].name
            if alloc.kind == "ExternalInput":
                if name != partition_name:
                    in_names.append(name)
            elif alloc.kind == "ExternalOutput":
                shape = tuple(alloc.tensor_shape)
                dtype = mybir.dt.np(alloc.dtype)
                out_names.append(name)
                out_avals.append(jax.core.ShapedArray(shape, dtype))
                zero_outs.append(np.zeros(shape, dtype))
        self.in_names, self.out_names = in_names, out_names
        self._out_avals, self._zero_outs = out_avals, zero_outs
        n_params = len(in_names)
        all_in = in_names + out_names + ([partition_name] if partition_name else [])

        def _body(*args):
            operands = list(args)
            if partition_name is not None:
                operands.append(partition_id_tensor())
            return tuple(_bass_exec_p.bind(
                *operands, out_avals=tuple(out_avals), in_names=tuple(all_in),
                out_names=tuple(out_names), lowering_input_output_aliases=(),
                sim_require_finite=True, sim_require_nnan=True, nc=nc))

        devices = jax.devices()[:n_cores]
        mesh = Mesh(np.asarray(devices), ("core",))
        nspec = n_params + len(out_names)
        self._fn = jax.jit(
            shard_map(_body, mesh=mesh, in_specs=(PartitionSpec("core"),) * nspec,
                      out_specs=(PartitionSpec("core"),) * len(out_names),
                      check_rep=False),
            keep_unused=True)

    def run(self, in_maps):
        n = self.n_cores
        per_core = [[np.ascontiguousarray(m[k]) for k in self.in_names] for m in in_maps]
        args = [np.concatenate([per_core[c][i] for c in range(n)], axis=0)
                for i in range(len(self.in_names))]
        args += [np.zeros((n * z.shape[0], *z.shape[1:]), z.dtype) for z in self._zero_outs]
        outs = self._fn(*args)
        self.jax.block_until_ready(outs)
        return [
            {name: np.asarray(outs[i]).reshape(n, *self._out_avals[i].shape)[c]
             for i, name in enumerate(self.out_names)}
            for c in range(n)
        ]


def _get_runner():
    global _RUNNER
    if _RUNNER is None:
        _RUNNER = _Runner(build_nc(), 8)
    return _RUNNER


def kernel(x, mask, ln1_g, ln1_b, qkv_w, qkv_b, proj_w, proj_b,
           ln2_g, ln2_b, fc1_w, fc1_b, fc2_w, fc2_b):
    x = np.asarray(x, np.float32)
    mask = np.asarray(mask, bool)
    ln1_g = np.asarray(ln1_g, np.float32); ln1_b = np.asarray(ln1_b, np.float32)
    qkv_w = np.asarray(qkv_w, np.float32); qkv_b = np.asarray(qkv_b, np.float32)
    proj_w = np.asarray(proj_w, np.float32); proj_b = np.asarray(proj_b, np.float32)
    ln2_g = np.asarray(ln2_g, np.float32); ln2_b = np.asarray(ln2_b, np.float32)
    fc1_w = np.asarray(fc1_w, np.float32); fc1_b = np.asarray(fc1_b, np.float32)
    fc2_w = np.asarray(fc2_w, np.float32); fc2_b = np.asarray(fc2_b, np.float32)

    # fold LN1 gain/bias + softmax scale into qkv weights/bias
    Wq = qkv_w * ln1_g[None, :]
    qb = qkv_w @ ln1_b + qkv_b
    Wq[:C] *= SCALE
    qb = qb.copy(); qb[:C] *= SCALE
    # qkv/proj weights in fp8e4m3 with a power-of-2 scale (undone by the
    # psum-read scale on device); activations h/oT are also fp8 so the
    # Q/K/V/proj matmuls run in DoubleRow mode at 2x PE rate.
    def _f8q(a, target=120.0):
        s = float(2.0 ** np.floor(np.log2(target / max(abs(a).max(), 1e-30))))
        q = (a * s).astype(ml_dtypes.float8_e4m3)
        assert np.isfinite(q.astype(np.float32)).all()
        return q, s
    qkv_wT, s_qkv = _f8q(np.ascontiguousarray(Wq.T))
    proj_wTt, s_p = _f8q(np.ascontiguousarray(
        proj_w.T.reshape(CT, P, CT, P).transpose(2, 1, 0, 3)))
    scales = np.zeros((P, 4), np.float32)
    scales[:, 0] = 1.0 / s_qkv          # q/k psum descale
    scales[:, 1] = 4.0 / s_qkv          # v psum descale * oT fp8 headroom
    scales[:, 2] = 1.0 / (4.0 * s_p)    # proj psum descale
    scales[:, 3] = 1.0
    # fold LN2 gain/bias into fc1
    W1 = fc1_w * ln2_g[None, :]
    fb1 = fc1_w @ ln2_b + fc1_b
    fc1_wT = np.ascontiguousarray(W1.T).astype(ml_dtypes.bfloat16)
    fc2_wTt = np.ascontiguousarray(
        fc2_w.T.reshape(HT, P, CT, P).transpose(2, 1, 0, 3)
    ).astype(ml_dtypes.bfloat16)
    qkvb_v = np.ascontiguousarray(
        (qb[2 * C:] * s_qkv).reshape(1, C)).astype(ml_dtypes.bfloat16)
    maskmul = (~mask).astype(ml_dtypes.bfloat16)

    in_maps = []
    for core in range(8):
        b, s = core // 2, core % 2
        # roll x so this core's query half is always tokens [0:512]
        xb = x[b]                         # [N, C]
        xroll = np.roll(xb, -s * NQ, axis=0)
        xT_c = np.ascontiguousarray(xroll.T)               # [C, NK]
        # mask rows q = this core's queries; key order must match rolled order
        mrow = maskmul[b, s * NQ:(s + 1) * NQ]             # [NQ, N] keys orig order
        mroll = np.roll(mrow, -s * NQ, axis=1)             # keys in rolled order
        maskT_c = np.ascontiguousarray(mroll.T)            # [NK, NQ]
        in_maps.append({
            "ones_d": np.ones(P, np.float32),
            "xT": xT_c.astype(np.float32),
            "maskT": maskT_c,
            "qkv_wT": qkv_wT, "qkvb_qk": qb[:2 * C].copy(), "qkvb_v": qkvb_v,
            "proj_wTt": proj_wTt, "projb": proj_b.copy(), "scales": scales,
            "fc1_wT": fc1_wT, "fc1b": fb1.copy(),
            "fc2_wTt": fc2_wTt, "fc2b": fc2_b.copy(),
        })

    results = _get_runner().run(in_maps)
    out = np.empty((B, N, C), np.float32)
    for core in range(8):
        b, s = core // 2, core % 2
        out[b, s * NQ:(s + 1) * NQ, :] = results[core]["outT"].T
    return out
